# revision 1
# baseline (speedup 1.0000x reference)
"""YOLOv1 loss kernel for Trainium2, 8-core data-parallel.

Strategy: shard batch (8192) across 8 cores (1024 each). Each core
processes its shard in NCHUNK chunks of G*128 batch items laid out as
[128 partitions, G, 1470] in SBUF (channel-major free dim). All box/IoU
arithmetic runs on the Vector engine (fused scalar_tensor_tensor ops
where possible); sqrt/square run on the Scalar (ACT) engine; DMAs on the
Sync (HWDGE) engine. Per-partition partial sums accumulate on-chip via
tensor_tensor_reduce; host sums the 8x128x3 partials and divides by B.

IoU is computed in cell-relative scaled coordinates: all three boxes of
a cell share the same (+m, +n)/G offset, so IoU is invariant to it, and
invariant to a uniform x-scale. With half-extents h = 3.5*w the overlap
width is min(2*ha, 2*hb, ha+hb-|dcx|) clamped at 0 (units: 7*w), and
union = 49*(wa*ha_frac...) i.e. 49*(w_a*h_a + w_g*h_g) - inter.
"""

import sys

import numpy as np

for _p in ("/opt/trn_rl_repo", "/root/.axon_site/_ro/trn_rl_repo"):
    if _p not in sys.path:
        sys.path.insert(0, _p)

import concourse.bass as bass
import concourse.mybir as mybir
from concourse.bass_utils import run_bass_kernel_spmd

F32 = mybir.dt.float32
U32 = mybir.dt.uint32
Alu = mybir.AluOpType
Act = mybir.ActivationFunctionType

B_TOTAL = 8192
NCORES = 8
B_CORE = B_TOTAL // NCORES  # 1024
P = 128
G = 2  # batch groups folded into the free dim per chunk
CHUNK = P * G  # 256
NCHUNK = B_CORE // CHUNK  # 4
C = 30
CELLS = 49
ROW = C * CELLS  # 1470


def build_nc(g: int = G, nchunk: int = NCHUNK):
    chunk = P * g
    nc = bass.Bass()
    pred = nc.declare_dram_parameter("pred", [B_CORE, ROW], F32, isOutput=False)
    labels = nc.declare_dram_parameter("labels", [B_CORE, ROW], F32, isOutput=False)
    out = nc.declare_dram_parameter("out", [P, 4], F32, isOutput=True)

    fshape = [P, g, CELLS]
    bshape = [P, g, 20 * CELLS]

    _ctr = [0]

    def sb(shape):
        _ctr[0] += 1
        return ctx_stack.enter_context(
            nc.sbuf_tensor(f"t{_ctr[0]}", shape, F32)
        )

    from contextlib import ExitStack

    ctx_stack = ExitStack()
    with ctx_stack:
        # double-buffered input tiles
        pt = [sb([P, g, ROW]) for _ in range(2)]
        lt = [sb([P, g, ROW]) for _ in range(2)]
        # ACT outputs (single-buffered; protected by sem schedule)
        sp2, sl2, sp3, sl3 = sb(fshape), sb(fshape), sb(fshape), sb(fshape)
        sp7, sl7, sp8, sl8 = sb(fshape), sb(fshape), sb(fshape), sb(fshape)
        q4, q9 = sb(fshape), sb(fshape)
        qx1, qy1, qx2, qy2 = sb(fshape), sb(fshape), sb(fshape), sb(fshape)
        e1, e2 = sb(fshape), sb(fshape)
        qsw1, qsh1, qsw2, qsh2 = sb(fshape), sb(fshape), sb(fshape), sb(fshape)
        sqcls = sb(bshape)
        # DVE temps
        dx1, dy1, dx2, dy2 = sb(fshape), sb(fshape), sb(fshape), sb(fshape)
        dx2c, dy2c = sb(fshape), sb(fshape)
        adx1, ady1, adx2, ady2 = sb(fshape), sb(fshape), sb(fshape), sb(fshape)
        sw1, sh1, sw2, sh2 = sb(fshape), sb(fshape), sb(fshape), sb(fshape)
        ta1, tb1, tc1 = sb(fshape), sb(fshape), sb(fshape)
        ta2, tb2, tc2 = sb(fshape), sb(fshape), sb(fshape)
        ta3, tb3, tc3 = sb(fshape), sb(fshape), sb(fshape)
        ta4, tb4, tc4 = sb(fshape), sb(fshape), sb(fshape)
        iw1, ih1, iw2, ih2 = sb(fshape), sb(fshape), sb(fshape), sb(fshape)
        int1, int2 = sb(fshape), sb(fshape)
        a1, a2, ag = sb(fshape), sb(fshape), sb(fshape)
        s1, s2 = sb(fshape), sb(fshape)
        u1, u2 = sb(fshape), sb(fshape)
        r1, r2 = sb(fshape), sb(fshape)
        iou1, iou2 = sb(fshape), sb(fshape)
        use1, objm = sb(fshape), sb(fshape)
        d1, d2 = sb(fshape), sb(fshape)
        dcls = sb(bshape)
        jbig = sb(bshape)
        dsw1, dsh1, dsw2, dsh2 = sb(fshape), sb(fshape), sb(fshape), sb(fshape)
        c1a, c1b, c2a, c2b = sb(fshape), sb(fshape), sb(fshape), sb(fshape)
        coor1, coor2 = sb(fshape), sb(fshape)
        de, hde, dc = sb(fshape), sb(fshape), sb(fshape)
        nc2t, mix, tsel = sb(fshape), sb(fshape), sb(fshape)
        clsum = sb(fshape)
        base, base2, base3, dd = sb(fshape), sb(fshape), sb(fshape), sb(fshape)
        junk = sb(fshape)
        junk2 = sb(fshape)
        red0 = ctx_stack.enter_context(nc.sbuf_tensor("red0", [P, 1], F32))
        red1 = ctx_stack.enter_context(nc.sbuf_tensor("red1", [P, 1], F32))
        acc = ctx_stack.enter_context(nc.sbuf_tensor("acc", [P, 4], F32))

        dma_sem = ctx_stack.enter_context(nc.semaphore("dma_sem"))
        sA1 = ctx_stack.enter_context(nc.semaphore("sA1"))
        sA2 = ctx_stack.enter_context(nc.semaphore("sA2"))
        sD1 = ctx_stack.enter_context(nc.semaphore("sD1"))
        sD2 = ctx_stack.enter_context(nc.semaphore("sD2"))
        v_done = ctx_stack.enter_context(nc.semaphore("v_done"))
        sGP = ctx_stack.enter_context(nc.semaphore("sGP"))
        block = ctx_stack.enter_context(nc.Block())

        def ch(t, c):  # channel slice -> [P, g, 49]
            return t[:, :, c * CELLS:(c + 1) * CELLS]

        def cls_blk(t):  # channels 10..29 -> [P, g, 980]
            return t[:, :, 10 * CELLS:30 * CELLS]

        @block.sync
        def _(sync):
            for i in range(nchunk):
                s = i % 2
                if i >= 1:
                    sync.wait_ge(dma_sem, 32 * i)
                if i >= 2:
                    sync.wait_ge(v_done, i - 1)
                rows = slice(i * chunk, (i + 1) * chunk)
                sync.dma_start(
                    out=pt[s][:],
                    in_=pred[rows].rearrange("(g p) d -> p g d", p=P),
                ).then_inc(dma_sem, 16)
                sync.dma_start(
                    out=lt[s][:],
                    in_=labels[rows].rearrange("(g p) d -> p g d", p=P),
                ).then_inc(dma_sem, 16)
            sync.wait_ge(v_done, nchunk)
            sync.dma_start(out=out[:], in_=acc[:]).then_inc(dma_sem, 16)
            sync.wait_ge(dma_sem, 32 * nchunk + 16)

        @block.gpsimd
        def _(gp):
            for i in range(nchunk):
                s = i % 2
                if i >= 1:
                    gp.wait_ge(v_done, i)
                gp.wait_ge(dma_sem, 32 * (i + 1))
                p, l = pt[s], lt[s]
                gp.tensor_tensor(dx2c[:], ch(p, 5), ch(l, 5), Alu.subtract)
                gp.tensor_tensor(dy2c[:], ch(p, 6), ch(l, 6), Alu.subtract)
                gp.tensor_scalar(objm[:], ch(l, 4), 1.0, None, Alu.is_equal)
                gp.tensor_tensor(dcls[:], cls_blk(p), cls_blk(l), Alu.subtract)
                gp.drain().then_inc(sGP, 1)

        @block.scalar
        def _(act):
            for i in range(nchunk):
                s = i % 2
                if i >= 1:
                    act.wait_ge(v_done, i)
                act.wait_ge(dma_sem, 32 * (i + 1))
                p, l = pt[s], lt[s]
                # phase 1: sqrts of w/h channels + conf squares
                act.activation(sp2[:], ch(p, 2), Act.Sqrt)
                act.activation(sl2[:], ch(l, 2), Act.Sqrt)
                act.activation(sp3[:], ch(p, 3), Act.Sqrt)
                act.activation(sl3[:], ch(l, 3), Act.Sqrt)
                act.activation(sp7[:], ch(p, 7), Act.Sqrt)
                act.activation(sl7[:], ch(l, 7), Act.Sqrt)
                act.activation(sp8[:], ch(p, 8), Act.Sqrt)
                act.activation(sl8[:], ch(l, 8), Act.Sqrt)
                act.activation(q4[:], ch(p, 4), Act.Square)
                act.activation(q9[:], ch(p, 9), Act.Square)
                act.drain().then_inc(sA1, 1)
                # phase 2a: squares of DVE diffs
                act.wait_ge(sD1, i + 1)
                act.activation(qx1[:], dx1[:], Act.Square)
                act.activation(qy1[:], dy1[:], Act.Square)
                act.activation(e1[:], d1[:], Act.Square)
                act.activation(e2[:], d2[:], Act.Square)
                act.wait_ge(sGP, i + 1)
                act.activation(qx2[:], dx2c[:], Act.Square)
                act.activation(qy2[:], dy2c[:], Act.Square)
                act.activation(sqcls[:], dcls[:], Act.Square)
                # phase 2b: squares of sqrt diffs
                act.wait_ge(sD2, i + 1)
                act.activation(qsw1[:], dsw1[:], Act.Square)
                act.activation(qsh1[:], dsh1[:], Act.Square)
                act.activation(qsw2[:], dsw2[:], Act.Square)
                act.activation(qsh2[:], dsh2[:], Act.Square)
                act.drain().then_inc(sA2, 1)

        @block.vector
        def _(v):
            stt = v.scalar_tensor_tensor
            tt = v.tensor_tensor
            ts = v.tensor_scalar

            v.memset(acc[:], 0.0)
            v.drain()
            for i in range(nchunk):
                s = i % 2
                v.wait_ge(dma_sem, 32 * (i + 1))
                p, l = pt[s], lt[s]
                # --- wave 1: direct from inputs ---
                tt(dx1[:], ch(p, 0), ch(l, 0), Alu.subtract)
                tt(dy1[:], ch(p, 1), ch(l, 1), Alu.subtract)
                tt(dx2[:], ch(p, 5), ch(l, 0), Alu.subtract)
                tt(dy2[:], ch(p, 6), ch(l, 1), Alu.subtract)
                tt(sw1[:], ch(p, 2), ch(l, 2), Alu.add)
                tt(sh1[:], ch(p, 3), ch(l, 3), Alu.add)
                tt(sw2[:], ch(p, 7), ch(l, 2), Alu.add)
                tt(sh2[:], ch(p, 8), ch(l, 3), Alu.add)
                tt(tc1[:], ch(p, 2), ch(l, 2), Alu.min)
                tt(tc2[:], ch(p, 3), ch(l, 3), Alu.min)
                tt(tc3[:], ch(p, 7), ch(l, 2), Alu.min)
                tt(tc4[:], ch(p, 8), ch(l, 3), Alu.min)
                tt(a1[:], ch(p, 2), ch(p, 3), Alu.mult)
                tt(a2[:], ch(p, 7), ch(p, 8), Alu.mult)
                tt(ag[:], ch(l, 2), ch(l, 3), Alu.mult)
                v.drain()
                # --- wave 2 ---
                ts(adx1[:].bitcast(U32), dx1[:].bitcast(U32), 0x7FFFFFFF, None,
                   Alu.bitwise_and)
                ts(ady1[:].bitcast(U32), dy1[:].bitcast(U32), 0x7FFFFFFF, None,
                   Alu.bitwise_and)
                ts(adx2[:].bitcast(U32), dx2[:].bitcast(U32), 0x7FFFFFFF, None,
                   Alu.bitwise_and)
                ts(ady2[:].bitcast(U32), dy2[:].bitcast(U32), 0x7FFFFFFF, None,
                   Alu.bitwise_and)
                tt(s1[:], a1[:], ag[:], Alu.add)
                tt(s2[:], a2[:], ag[:], Alu.add)
                v.drain()
                # --- wave 3: overlap = min(S-|d|, 7wa, 7wb), clamped ---
                stt(ta1[:], sw1[:], 3.5, adx1[:], Alu.mult, Alu.subtract)
                stt(ta2[:], sh1[:], 3.5, ady1[:], Alu.mult, Alu.subtract)
                stt(ta3[:], sw2[:], 3.5, adx2[:], Alu.mult, Alu.subtract)
                stt(ta4[:], sh2[:], 3.5, ady2[:], Alu.mult, Alu.subtract)
                v.drain()
                # --- wave 4: min vs 7*min(wa,wb) ---
                stt(tb1[:], tc1[:], 7.0, ta1[:], Alu.mult, Alu.min)
                stt(tb2[:], tc2[:], 7.0, ta2[:], Alu.mult, Alu.min)
                stt(tb3[:], tc3[:], 7.0, ta3[:], Alu.mult, Alu.min)
                stt(tb4[:], tc4[:], 7.0, ta4[:], Alu.mult, Alu.min)
                v.drain()
                # --- wave 5: clamp ---
                ts(iw1[:], tb1[:], 0.0, None, Alu.max)
                ts(ih1[:], tb2[:], 0.0, None, Alu.max)
                ts(iw2[:], tb3[:], 0.0, None, Alu.max)
                ts(ih2[:], tb4[:], 0.0, None, Alu.max)
                v.drain()
                # --- wave 7 ---
                tt(int1[:], iw1[:], ih1[:], Alu.mult)
                tt(int2[:], iw2[:], ih2[:], Alu.mult)
                v.drain()
                # --- wave 8: union = 49*(area_p + area_g) - inter ---
                stt(u1[:], s1[:], 49.0, int1[:], Alu.mult, Alu.subtract)
                stt(u2[:], s2[:], 49.0, int2[:], Alu.mult, Alu.subtract)
                v.drain()
                # --- wave 9 ---
                v.reciprocal(r1[:], u1[:])
                v.reciprocal(r2[:], u2[:])
                v.drain()
                # --- wave 10 ---
                tt(iou1[:], int1[:], r1[:], Alu.mult)
                tt(iou2[:], int2[:], r2[:], Alu.mult)
                v.drain()
                # --- wave 11 ---
                tt(use1[:], iou1[:], iou2[:], Alu.is_ge)
                tt(d1[:], ch(p, 4), iou1[:], Alu.subtract)
                tt(d2[:], ch(p, 9), iou2[:], Alu.subtract)
                v.drain().then_inc(sD1, 1)
                # --- wave 12: sqrt diffs (needs ACT phase 1) ---
                v.wait_ge(sA1, i + 1)
                tt(dsw1[:], sp2[:], sl2[:], Alu.subtract)
                tt(dsh1[:], sp3[:], sl3[:], Alu.subtract)
                tt(dsw2[:], sp7[:], sl7[:], Alu.subtract)
                tt(dsh2[:], sp8[:], sl8[:], Alu.subtract)
                v.drain().then_inc(sD2, 1)
                # --- wave 13+: combine (needs ACT phase 2) ---
                v.wait_ge(sA2, i + 1)
                v.tensor_reduce(
                    out=clsum[:],
                    in_=sqcls[:].rearrange("p g (c k) -> p g k c", c=20),
                    axis=mybir.AxisListType.X, op=Alu.add,
                )
                tt(de[:], e1[:], e2[:], Alu.subtract)
                tt(nc2t[:], q4[:], q9[:], Alu.add)
                tt(c1a[:], qx1[:], qy1[:], Alu.add)
                tt(c1b[:], qsw1[:], qsh1[:], Alu.add)
                tt(c2a[:], qx2[:], qy2[:], Alu.add)
                tt(c2b[:], qsw2[:], qsh2[:], Alu.add)
                v.drain()
                tt(coor1[:], c1a[:], c1b[:], Alu.add)
                tt(coor2[:], c2a[:], c2b[:], Alu.add)
                ts(hde[:], de[:], 0.5, None, Alu.mult)
                v.drain()
                tt(dc[:], coor1[:], coor2[:], Alu.subtract)
                stt(base[:], coor2[:], 5.0, e2[:], Alu.mult, Alu.add)
                v.drain()
                stt(mix[:], dc[:], 5.0, hde[:], Alu.mult, Alu.add)
                stt(base2[:], e1[:], 0.5, base[:], Alu.mult, Alu.add)
                v.drain()
                tt(tsel[:], use1[:], mix[:], Alu.mult)
                tt(junk2[:], clsum[:], base2[:], Alu.add)
                v.drain()
                tt(base3[:], junk2[:], tsel[:], Alu.add)
                v.drain()
                stt(dd[:], nc2t[:], -0.5, base3[:], Alu.mult, Alu.add)
                v.drain()
                # accumulate: acc0 += sum(obj * dd); acc1 += 0.5*sum(nc2)
                tt(junk[:], objm[:], dd[:], Alu.mult)
                v.drain()
                v.tensor_reduce(out=red0[:], in_=junk[:],
                                axis=mybir.AxisListType.XY, op=Alu.add)
                v.tensor_reduce(out=red1[:], in_=nc2t[:],
                                axis=mybir.AxisListType.XY, op=Alu.add)
                v.drain()
                stt(acc[:, 0:1], red0[:], 1.0, acc[:, 0:1], Alu.mult, Alu.add)
                stt(acc[:, 1:2], red1[:], 0.5, acc[:, 1:2], Alu.mult, Alu.add)
                v.drain().then_inc(v_done, 1)

    return nc


_NC_CACHE = {}


def _get_nc():
    if "nc" not in _NC_CACHE:
        _NC_CACHE["nc"] = build_nc()
    return _NC_CACHE["nc"]


def run_device(pred, labels, trace=False):
    nc = _get_nc()
    pred = np.ascontiguousarray(pred, dtype=np.float32).reshape(B_TOTAL, ROW)
    labels = np.ascontiguousarray(labels, dtype=np.float32).reshape(B_TOTAL, ROW)
    in_maps = []
    for c in range(NCORES):
        rows = slice(c * B_CORE, (c + 1) * B_CORE)
        in_maps.append({"pred": pred[rows], "labels": labels[rows]})
    res = run_bass_kernel_spmd(nc, in_maps, list(range(NCORES)), trace=trace)
    total = 0.0
    for c in range(NCORES):
        total += float(res.results[c]["out"][:, :3].astype(np.float64).sum())
    loss = np.float32(total / B_TOTAL)
    return loss, res


def kernel(pred, labels):
    loss, _ = run_device(pred, labels, trace=False)
    return np.array(loss, dtype=np.float32)


if __name__ == "__main__":
    rng = np.random.default_rng(0)
    p = rng.random((B_TOTAL, C, 7, 7), dtype=np.float32)
    l = rng.random((B_TOTAL, C, 7, 7), dtype=np.float32)
    l[:, 4] = (rng.random((B_TOTAL, 7, 7)) < 0.3).astype(np.float32)
    print(kernel(p, l))



# revision 18
# speedup vs baseline: 1.8444x; 1.8444x over previous
"""YOLOv1 loss kernel for Trainium2, 8-core data-parallel, bf16 pipeline.

Strategy: shard batch (8192) across 8 cores (1024 rows each). Host converts
inputs to bf16 (labels obj channel converted equality-preserving so l4 == 1.0
stays exact) and repacks channels per-row so every multi-channel device op is
one contiguous instruction:

  pred row (30 ch):  [0,1,5,6 | 2,3,7,8 | 4,9 | 10..29]
  label row (35 ch): [0,1,0,1 | 2,3,2,3 | 5,6 | 2,3,7,8 | 4 | 10..29]

Each core streams its shard in uneven chunks (g units of 128 rows, layout
[128, g, ch, 49]). Per chunk the whole loss reduces to ONE Scalar-engine
Square+accumulate over a packed masked buffer mball[g, 32, 49]:
  slots 0:8   sqrt(5*om_b) * (coor diffs: dx, dy, sqrt-w, sqrt-h per box)
  slots 8:10  sqrt(mA/mB) * (conf - iou) per box
  slots 10:12 sqrt(0.5*(1-obj)) * (p4, p9)
  slots 12:32 obj * (pred_cls - label_cls)
where om_u = obj*use1, om_nu = obj*(1-use1), mA = om_u + 0.5*om_nu,
mB = om_nu + 0.5*om_u. sum(mball^2) == chunk loss contribution exactly.

IoU runs in 7x-scaled units: ov7 = max(min(7*min(w), 3.5*(wp+wl) - |dc|), 0),
ints49 = ov7w*ov7h, u49 = 49*(area_p + area_g) - ints49, iou = ints49/u49.

Engine split: DVE does diffs/IoU (tensor_tensor 2x bf16, tensor_scalar 4x)
with the masked-multiply wave lagged one chunk behind (software pipelining);
Pool (gpsimd) builds the mask vector in 3 ops plus the small masked conf/q
multiplies; ACT does sqrts, one Sqrt(scale=5) mask-root op and the final
Square+accum. Out: acc[128, NCHUNK] fp32 per core, summed on host in fp64.
"""

import sys

import numpy as np

for _p in ("/opt/trn_rl_repo", "/root/.axon_site/_ro/trn_rl_repo"):
    if _p not in sys.path:
        sys.path.insert(0, _p)

import concourse.bass as bass
import concourse.mybir as mybir
from concourse.bass_utils import run_bass_kernel_spmd

F32 = mybir.dt.float32
BF16 = mybir.dt.bfloat16
U16 = mybir.dt.uint16
Alu = mybir.AluOpType
Act = mybir.ActivationFunctionType

B_TOTAL = 8192
NCORES = 8
B_CORE = B_TOTAL // NCORES  # 1024
P = 128
C = 30
K = 49
CP = 30   # repacked pred channels
CL = 35   # repacked label channels
ROWP = CP * K
ROWL = CL * K

# host channel permutations
PP_IDX = [0, 1, 5, 6, 2, 3, 7, 8, 4, 9] + list(range(10, 30))
LL_IDX = [0, 1, 0, 1, 2, 3, 2, 3, 5, 6, 2, 3, 7, 8, 4] + list(range(10, 30))

CHUNKS = (2, 3, 2, 1)


def build_nc(chunks=CHUNKS):
    assert sum(chunks) * P == B_CORE
    nchunk = len(chunks)
    maxg = max(chunks)
    nc = bass.Bass()
    pred = nc.declare_dram_parameter("pred", [B_CORE, ROWP], BF16, isOutput=False)
    labels = nc.declare_dram_parameter("labels", [B_CORE, ROWL], BF16,
                                       isOutput=False)
    out = nc.declare_dram_parameter("out", [P, nchunk], F32, isOutput=True)

    from contextlib import ExitStack

    _ctr = [0]
    es = ExitStack()

    def sb(shape, dt=BF16):
        _ctr[0] += 1
        return es.enter_context(nc.sbuf_tensor(f"t{_ctr[0]}", shape, dt))

    with es:
        pt = [sb([P, maxg, CP, K]) for _ in range(2)]
        lt = [sb([P, maxg, CL, K]) for _ in range(2)]
        # dxyb slots: 0:2 b2-iou-xy, 2:4 b1-xy, 4:6 b1-sqrt, 6:8 b2-xy, 8:10 b2-sqrt
        dxyb = [sb([P, maxg, 10, K]) for _ in range(2)]
        adb = [sb([P, maxg, 4, K]) for _ in range(2)]     # |b2Ix,b2Iy,b1x,b1y|
        swh = [sb([P, maxg, 2, 2, K]) for _ in range(2)]
        s35 = [sb([P, maxg, 2, 2, K]) for _ in range(2)]
        mwh = [sb([P, maxg, 2, 2, K]) for _ in range(2)]
        mwh7 = [sb([P, maxg, 2, 2, K]) for _ in range(2)]
        ta = [sb([P, maxg, 2, 2, K]) for _ in range(2)]
        ov = [sb([P, maxg, 2, 2, K]) for _ in range(2)]
        cl = [sb([P, maxg, 2, 2, K]) for _ in range(2)]
        ints = [sb([P, maxg, 2, K]) for _ in range(2)]
        apw = [sb([P, maxg, 2, K]) for _ in range(2)]
        agb = [sb([P, maxg, 1, K]) for _ in range(2)]
        sa = [sb([P, maxg, 2, K]) for _ in range(2)]
        sa49 = [sb([P, maxg, 2, K]) for _ in range(2)]
        u49 = [sb([P, maxg, 2, K]) for _ in range(2)]
        rcp = [sb([P, maxg, 2, K]) for _ in range(2)]
        iou = [sb([P, maxg, 2, K]) for _ in range(2)]
        tq = [sb([P, maxg, 4, K]) for _ in range(2)]      # use1, t3, t1, t2
        dconf = [sb([P, maxg, 2, K]) for _ in range(2)]
        dcls = [sb([P, maxg, 20, K]) for _ in range(2)]
        sqp = [sb([P, maxg, 2, 2, K]) for _ in range(2)]
        sql = [sb([P, maxg, 2, 2, K]) for _ in range(2)]
        objm = [sb([P, maxg, K]) for _ in range(2)]
        rm = [sb([P, maxg, 5, K]) for _ in range(2)]
        mball = [sb([P, maxg, 32, K]) for _ in range(2)]
        junk32 = [sb([P, maxg, 32, K]) for _ in range(2)]
        acc = es.enter_context(nc.sbuf_tensor("acc", [P, nchunk], F32))

        dsemA = es.enter_context(nc.semaphore("dsemA"))
        dsemB = es.enter_context(nc.semaphore("dsemB"))
        dsems = [dsemA, dsemB]
        u_done = es.enter_context(nc.semaphore("u_done"))
        sqrt_done = es.enter_context(nc.semaphore("sqrt_done"))
        rm_done = es.enter_context(nc.semaphore("rm_done"))
        mball_dve = es.enter_context(nc.semaphore("mball_dve"))
        mball_pool = es.enter_context(nc.semaphore("mball_pool"))
        acc_done = es.enter_context(nc.semaphore("acc_done"))
        tfree_dve = es.enter_context(nc.semaphore("tfree_dve"))
        block = es.enter_context(nc.Block())

        offs = [0]
        for g in chunks:
            offs.append(offs[-1] + g * P)

        @block.sync
        def _(sync):
            for i, g in enumerate(chunks):
                s = i % 2
                if i >= 2:
                    sync.wait_ge(sqrt_done, i - 1)
                    sync.wait_ge(mball_pool, i - 1)
                    sync.wait_ge(tfree_dve, i - 1)
                rows = slice(offs[i], offs[i + 1])
                sync.dma_start(
                    out=pt[s][:, 0:g].rearrange("p g c k -> p g (c k)"),
                    in_=pred[rows].rearrange("(g p) d -> p g d", p=P),
                ).then_inc(dsems[s], 16)
                sync.dma_start(
                    out=lt[s][:, 0:g].rearrange("p g c k -> p g (c k)"),
                    in_=labels[rows].rearrange("(g p) d -> p g d", p=P),
                ).then_inc(dsems[s], 16)
            sync.wait_ge(acc_done, nchunk)
            sync.dma_start(out=out[:], in_=acc[:]).then_inc(dsemA, 16)
            sync.wait_ge(dsemA, 32 * ((nchunk + 1) // 2) + 16)

        @block.gpsimd
        def _(gp):
            for i, g in enumerate(chunks):
                s = i % 2
                gp.wait_ge(dsems[s], 32 * (i // 2 + 1))
                if i >= 2:
                    gp.wait_ge(mball_dve, i - 1)
                gp.tensor_scalar(objm[s][:, 0:g], lt[s][:, 0:g, 14:15, :],
                                 1.0, None, Alu.is_equal)
                gp.drain()
                gp.wait_ge(u_done, i + 1)
                gp.tensor_scalar(rm[s][:, 0:g, 4:5, :], objm[s][:, 0:g],
                                 -0.70710678, 0.70710678, Alu.mult, Alu.add)
                gp.tensor_tensor(
                    rm[s][:, 0:g, 0:4, :],
                    objm[s][:, 0:g].unsqueeze(2).broadcast_to([P, g, 4, K]),
                    tq[s][:, 0:g], Alu.mult,
                ).then_inc(rm_done, 1)
                gp.drain()
                if i >= 2:
                    gp.wait_ge(acc_done, i - 1)
                gp.tensor_tensor(
                    mball[s][:, 0:g, 10:12, :], pt[s][:, 0:g, 8:10, :],
                    rm[s][:, 0:g, 4:5, :].broadcast_to([P, g, 2, K]),
                    Alu.mult,
                )
                gp.tensor_tensor(
                    mball[s][:, 0:g, 8:10, :], dconf[s][:, 0:g],
                    rm[s][:, 0:g, 2:4, :], Alu.mult,
                ).then_inc(mball_pool, 1)

        @block.scalar
        def _(act):
            def sq_acc(j):
                sj = j % 2
                gj = chunks[j]
                act.wait_ge(mball_dve, j + 1)
                act.wait_ge(mball_pool, j + 1)
                act.activation(
                    junk32[sj][:, 0:gj].rearrange("p g c k -> p (g c k)"),
                    mball[sj][:, 0:gj].rearrange("p g c k -> p (g c k)"),
                    Act.Square,
                    accum_out=acc[:, j:j + 1],
                ).then_inc(acc_done, 1)

            for i, g in enumerate(chunks):
                s = i % 2
                act.wait_ge(dsems[s], 32 * (i // 2 + 1))
                act.activation(sqp[s][:, 0:g], pt[s][:, 0:g, 4:8, :], Act.Sqrt)
                act.activation(sql[s][:, 0:g], lt[s][:, 0:g, 10:14, :],
                               Act.Sqrt).then_inc(sqrt_done, 1)
                if i >= 2:
                    sq_acc(i - 2)
            sq_acc(nchunk - 2)
            sq_acc(nchunk - 1)

        @block.vector
        def _(v):
            tt = v.tensor_tensor
            ts = v.tensor_scalar

            def lagged_mults(j):
                sj = j % 2
                gj = chunks[j]
                v.wait_ge(rm_done, j + 1)
                if j >= 2:
                    v.wait_ge(acc_done, j - 1)
                tt(mball[sj][:, 0:gj, 0:4, :], dxyb[sj][:, 0:gj, 2:6, :],
                   rm[sj][:, 0:gj, 0:1, :].broadcast_to([P, gj, 4, K]),
                   Alu.mult)
                tt(mball[sj][:, 0:gj, 4:8, :], dxyb[sj][:, 0:gj, 6:10, :],
                   rm[sj][:, 0:gj, 1:2, :].broadcast_to([P, gj, 4, K]),
                   Alu.mult)
                tt(mball[sj][:, 0:gj, 12:32, :], dcls[sj][:, 0:gj],
                   objm[sj][:, 0:gj].unsqueeze(2).broadcast_to([P, gj, 20, K]),
                   Alu.mult).then_inc(mball_dve, 1)
                v.drain()

            for i, g in enumerate(chunks):
                s = i % 2
                p, l = pt[s], lt[s]
                v.wait_ge(dsems[s], 32 * (i // 2 + 1))
                # W1: reads only tiles
                tt(dxyb[s][:, 0:g, 2:4, :], p[:, 0:g, 0:2, :],
                   l[:, 0:g, 0:2, :], Alu.subtract)
                tt(dxyb[s][:, 0:g, 0:2, :], p[:, 0:g, 2:4, :],
                   l[:, 0:g, 2:4, :], Alu.subtract)
                tt(dxyb[s][:, 0:g, 6:8, :], p[:, 0:g, 2:4, :],
                   l[:, 0:g, 8:10, :], Alu.subtract)
                tt(swh[s][:, 0:g], p[:, 0:g, 4:8, :], l[:, 0:g, 4:8, :],
                   Alu.add)
                tt(mwh[s][:, 0:g], p[:, 0:g, 4:8, :], l[:, 0:g, 4:8, :],
                   Alu.min)
                tt(apw[s][:, 0:g],
                   p[:, 0:g, 4:8, :].rearrange("p g (b w) k -> p g b w k",
                                               b=2)[:, :, :, 0, :],
                   p[:, 0:g, 4:8, :].rearrange("p g (b w) k -> p g b w k",
                                               b=2)[:, :, :, 1, :],
                   Alu.mult)
                tt(agb[s][:, 0:g], l[:, 0:g, 4:5, :], l[:, 0:g, 5:6, :],
                   Alu.mult)
                tt(dcls[s][:, 0:g], p[:, 0:g, 10:30, :], l[:, 0:g, 15:35, :],
                   Alu.subtract)
                v.drain()
                # W2
                ts(adb[s][:, 0:g, 0:2, :].bitcast(U16),
                   dxyb[s][:, 0:g, 2:4, :].bitcast(U16),
                   0x7FFF, None, Alu.bitwise_and)
                ts(adb[s][:, 0:g, 2:4, :].bitcast(U16),
                   dxyb[s][:, 0:g, 0:2, :].bitcast(U16),
                   0x7FFF, None, Alu.bitwise_and)
                ts(s35[s][:, 0:g], swh[s][:, 0:g], 3.5, None, Alu.mult)
                ts(mwh7[s][:, 0:g], mwh[s][:, 0:g], 7.0, None, Alu.mult)
                tt(sa[s][:, 0:g], apw[s][:, 0:g],
                   agb[s][:, 0:g].broadcast_to([P, g, 2, K]), Alu.add)
                v.drain()
                # W3  (adb is box-reversed relative to s35: flip its view)
                tt(ta[s][:, 0:g], s35[s][:, 0:g],
                   adb[s][:, 0:g].rearrange("p g (a c) k -> p g a c k", a=2),
                   Alu.subtract)
                ts(sa49[s][:, 0:g], sa[s][:, 0:g], 49.0, None, Alu.mult)
                v.drain()
                tt(ov[s][:, 0:g], mwh7[s][:, 0:g], ta[s][:, 0:g], Alu.min)
                v.drain()
                ts(cl[s][:, 0:g], ov[s][:, 0:g], 0.0, None, Alu.max)
                v.drain()
                tt(ints[s][:, 0:g], cl[s][:, 0:g, :, 0:1, :],
                   cl[s][:, 0:g, :, 1:2, :], Alu.mult)
                v.drain()
                tt(u49[s][:, 0:g], sa49[s][:, 0:g], ints[s][:, 0:g],
                   Alu.subtract)
                v.drain()
                with nc.allow_low_precision(reason="bf16 iou tolerated"):
                    v.reciprocal(rcp[s][:, 0:g], u49[s][:, 0:g])
                v.drain()
                tt(iou[s][:, 0:g], ints[s][:, 0:g], rcp[s][:, 0:g], Alu.mult)
                v.drain()
                # W10: use1 + dconf
                tt(tq[s][:, 0:g, 0:1, :], iou[s][:, 0:g, 0:1, :],
                   iou[s][:, 0:g, 1:2, :], Alu.is_ge)
                tt(dconf[s][:, 0:g], p[:, 0:g, 8:10, :], iou[s][:, 0:g],
                   Alu.subtract)
                v.drain()
                # W10b: tq = linear sqrt-mask factors of u ; dsq
                ts(tq[s][:, 0:g, 1:2, :], tq[s][:, 0:g, 0:1, :],
                   -2.23606798, 2.23606798, Alu.mult, Alu.add)
                ts(tq[s][:, 0:g, 2:3, :], tq[s][:, 0:g, 0:1, :],
                   0.29289322, 0.70710678, Alu.mult, Alu.add)
                ts(tq[s][:, 0:g, 3:4, :], tq[s][:, 0:g, 0:1, :],
                   -0.29289322, 1.0, Alu.mult, Alu.add)
                ts(tq[s][:, 0:g, 0:1, :], tq[s][:, 0:g, 0:1, :],
                   2.23606798, None, Alu.mult)
                v.wait_ge(sqrt_done, i + 1)
                tt(dxyb[s][:, 0:g, 4:6, :], sqp[s][:, 0:g, 0:1, :, :],
                   sql[s][:, 0:g, 0:1, :, :], Alu.subtract)
                tt(dxyb[s][:, 0:g, 8:10, :], sqp[s][:, 0:g, 1:2, :, :],
                   sql[s][:, 0:g, 1:2, :, :],
                   Alu.subtract).then_inc(u_done, 1)
                v.drain().then_inc(tfree_dve, 1)
                if i >= 1:
                    lagged_mults(i - 1)
            lagged_mults(nchunk - 1)

    return nc


_NC_CACHE = {}


def _get_nc():
    if "nc" not in _NC_CACHE:
        _NC_CACHE["nc"] = build_nc()
    return _NC_CACHE["nc"]


def _to_bf16_repack(pred, labels):
    import ml_dtypes

    bf = ml_dtypes.bfloat16
    p = np.ascontiguousarray(pred, dtype=np.float32).reshape(B_TOTAL, C, K)
    l = np.ascontiguousarray(labels, dtype=np.float32).reshape(B_TOTAL, C, K)
    pb = p.astype(bf)
    lb = l.astype(bf)
    # obj channel: keep the ==1.0 test exact under rounding
    l4 = l[:, 4, :]
    lb4 = lb[:, 4, :]
    bad = (l4 != np.float32(1.0)) & (lb4.astype(np.float32) == np.float32(1.0))
    if bad.any():
        lb4[bad] = bf(0.99609375)
        lb[:, 4, :] = lb4
    prp = np.ascontiguousarray(pb[:, PP_IDX, :]).reshape(B_TOTAL, ROWP)
    lrp = np.ascontiguousarray(lb[:, LL_IDX, :]).reshape(B_TOTAL, ROWL)
    return prp, lrp


def run_device(pred, labels, trace=False):
    nc = _get_nc()
    prp, lrp = _to_bf16_repack(pred, labels)
    in_maps = []
    for c in range(NCORES):
        rows = slice(c * B_CORE, (c + 1) * B_CORE)
        in_maps.append({"pred": prp[rows], "labels": lrp[rows]})
    res = run_bass_kernel_spmd(nc, in_maps, list(range(NCORES)), trace=trace)
    total = 0.0
    for c in range(NCORES):
        total += float(res.results[c]["out"].astype(np.float64).sum())
    loss = np.float32(total / B_TOTAL)
    return loss, res


def kernel(pred, labels):
    loss, _ = run_device(pred, labels, trace=False)
    return np.array(loss, dtype=np.float32)


if __name__ == "__main__":
    rng = np.random.default_rng(0)
    p = rng.random((B_TOTAL, C, 7, 7), dtype=np.float32)
    l = rng.random((B_TOTAL, C, 7, 7), dtype=np.float32)
    l[:, 4] = (rng.random((B_TOTAL, 7, 7)) < 0.3).astype(np.float32)
    print(kernel(p, l))


# revision 19
# speedup vs baseline: 1.9093x; 1.0352x over previous
"""YOLOv1 loss kernel for Trainium2, 8-core data-parallel, bf16 pipeline.

Strategy: shard batch (8192) across 8 cores (1024 rows each). Host converts
inputs to bf16 (labels obj channel converted equality-preserving so l4 == 1.0
stays exact) and repacks channels per-row so every multi-channel device op is
one contiguous instruction:

  pred row (30 ch):  [0,1,5,6 | 2,3,7,8 | 4,9 | 10..29]
  label row (35 ch): [0,1,0,1 | 2,3,2,3 | 5,6 | 2,3,7,8 | 4 | 10..29]

Each core streams its shard in uneven chunks (g units of 128 rows, layout
[128, g, ch, 49]). Per chunk the whole loss reduces to ONE Scalar-engine
Square+accumulate over a packed masked buffer mball[g, 32, 49]:
  slots 0:8   sqrt(5*om_b) * (coor diffs: dx, dy, sqrt-w, sqrt-h per box)
  slots 8:10  sqrt(mA/mB) * (conf - iou) per box
  slots 10:12 sqrt(0.5*(1-obj)) * (p4, p9)
  slots 12:32 obj * (pred_cls - label_cls)
where om_u = obj*use1, om_nu = obj*(1-use1), mA = om_u + 0.5*om_nu,
mB = om_nu + 0.5*om_u. sum(mball^2) == chunk loss contribution exactly.

IoU runs in 7x-scaled units: ov7 = max(min(7*min(w), 3.5*(wp+wl) - |dc|), 0),
ints49 = ov7w*ov7h, u49 = 49*(area_p + area_g) - ints49, iou = ints49/u49.

Engine split: DVE does diffs/IoU (tensor_tensor 2x bf16, tensor_scalar 4x)
with the masked-multiply wave lagged one chunk behind (software pipelining);
Pool (gpsimd) builds the mask vector in 3 ops plus the small masked conf/q
multiplies; ACT does sqrts, one Sqrt(scale=5) mask-root op and the final
Square+accum. Out: acc[128, NCHUNK] fp32 per core, summed on host in fp64.
"""

import sys

import numpy as np

for _p in ("/opt/trn_rl_repo", "/root/.axon_site/_ro/trn_rl_repo"):
    if _p not in sys.path:
        sys.path.insert(0, _p)

import concourse.bass as bass
import concourse.mybir as mybir
from concourse.bass_utils import run_bass_kernel_spmd

F32 = mybir.dt.float32
BF16 = mybir.dt.bfloat16
U16 = mybir.dt.uint16
Alu = mybir.AluOpType
Act = mybir.ActivationFunctionType

B_TOTAL = 8192
NCORES = 8
B_CORE = B_TOTAL // NCORES  # 1024
P = 128
C = 30
K = 49
CP = 30   # repacked pred channels
CL = 35   # repacked label channels
ROWP = CP * K
ROWL = CL * K

# host channel permutations
PP_IDX = [0, 1, 5, 6, 2, 3, 7, 8, 4, 9] + list(range(10, 30))
LL_IDX = [0, 1, 0, 1, 2, 3, 2, 3, 5, 6, 2, 3, 7, 8, 4] + list(range(10, 30))

CHUNKS = (2, 3, 2, 1)


def build_nc(chunks=CHUNKS):
    assert sum(chunks) * P == B_CORE
    nchunk = len(chunks)
    maxg = max(chunks)
    nc = bass.Bass()
    pred = nc.declare_dram_parameter("pred", [B_CORE, ROWP], BF16, isOutput=False)
    labels = nc.declare_dram_parameter("labels", [B_CORE, ROWL], BF16,
                                       isOutput=False)
    out = nc.declare_dram_parameter("out", [P, nchunk], F32, isOutput=True)

    from contextlib import ExitStack

    _ctr = [0]
    es = ExitStack()

    def sb(shape, dt=BF16):
        _ctr[0] += 1
        return es.enter_context(nc.sbuf_tensor(f"t{_ctr[0]}", shape, dt))

    with es:
        pt = [sb([P, maxg, CP, K]) for _ in range(2)]
        lt = [sb([P, maxg, CL, K]) for _ in range(2)]
        # dxyb slots: 0:2 b2-iou-xy, 2:4 b1-xy, 4:6 b1-sqrt, 6:8 b2-xy, 8:10 b2-sqrt
        dxyb = [sb([P, maxg, 10, K]) for _ in range(2)]
        adb = [sb([P, maxg, 4, K]) for _ in range(2)]     # |b2Ix,b2Iy,b1x,b1y|
        swh = [sb([P, maxg, 2, 2, K]) for _ in range(2)]
        s35 = [sb([P, maxg, 2, 2, K]) for _ in range(2)]
        mwh = [sb([P, maxg, 2, 2, K]) for _ in range(2)]
        mwh7 = [sb([P, maxg, 2, 2, K]) for _ in range(2)]
        ta = [sb([P, maxg, 2, 2, K]) for _ in range(2)]
        ov = [sb([P, maxg, 2, 2, K]) for _ in range(2)]
        cl = [sb([P, maxg, 2, 2, K]) for _ in range(2)]
        ints = [sb([P, maxg, 2, K]) for _ in range(2)]
        apw = [sb([P, maxg, 2, K]) for _ in range(2)]
        agb = [sb([P, maxg, 1, K]) for _ in range(2)]
        sa = [sb([P, maxg, 2, K]) for _ in range(2)]
        sa49 = [sb([P, maxg, 2, K]) for _ in range(2)]
        u49 = [sb([P, maxg, 2, K]) for _ in range(2)]
        rcp = [sb([P, maxg, 2, K]) for _ in range(2)]
        iou = [sb([P, maxg, 2, K]) for _ in range(2)]
        tq = [sb([P, maxg, 4, K]) for _ in range(2)]      # use1, t3, t1, t2
        dconf = [sb([P, maxg, 2, K]) for _ in range(2)]
        dcls = [sb([P, maxg, 14, K]) for _ in range(2)]
        dclsp = [sb([P, maxg, 6, K]) for _ in range(2)]
        sqp = [sb([P, maxg, 2, 2, K]) for _ in range(2)]
        sql = [sb([P, maxg, 2, 2, K]) for _ in range(2)]
        objm = [sb([P, maxg, K]) for _ in range(2)]
        rm = [sb([P, maxg, 5, K]) for _ in range(2)]
        mball = [sb([P, maxg, 32, K]) for _ in range(2)]
        junk32 = [sb([P, maxg, 32, K]) for _ in range(2)]
        acc = es.enter_context(nc.sbuf_tensor("acc", [P, nchunk], F32))

        dsemA = es.enter_context(nc.semaphore("dsemA"))
        dsemB = es.enter_context(nc.semaphore("dsemB"))
        dsems = [dsemA, dsemB]
        u_done = es.enter_context(nc.semaphore("u_done"))
        sqrt_done = es.enter_context(nc.semaphore("sqrt_done"))
        rm_done = es.enter_context(nc.semaphore("rm_done"))
        mball_dve = es.enter_context(nc.semaphore("mball_dve"))
        mball_pool = es.enter_context(nc.semaphore("mball_pool"))
        acc_done = es.enter_context(nc.semaphore("acc_done"))
        tfree_dve = es.enter_context(nc.semaphore("tfree_dve"))
        block = es.enter_context(nc.Block())

        offs = [0]
        for g in chunks:
            offs.append(offs[-1] + g * P)

        @block.sync
        def _(sync):
            for i, g in enumerate(chunks):
                s = i % 2
                if i >= 2:
                    sync.wait_ge(sqrt_done, i - 1)
                    sync.wait_ge(mball_pool, i - 1)
                    sync.wait_ge(tfree_dve, i - 1)
                rows = slice(offs[i], offs[i + 1])
                sync.dma_start(
                    out=pt[s][:, 0:g].rearrange("p g c k -> p g (c k)"),
                    in_=pred[rows].rearrange("(g p) d -> p g d", p=P),
                ).then_inc(dsems[s], 16)
                sync.dma_start(
                    out=lt[s][:, 0:g].rearrange("p g c k -> p g (c k)"),
                    in_=labels[rows].rearrange("(g p) d -> p g d", p=P),
                ).then_inc(dsems[s], 16)
            sync.wait_ge(acc_done, nchunk)
            sync.dma_start(out=out[:], in_=acc[:]).then_inc(dsemA, 16)
            sync.wait_ge(dsemA, 32 * ((nchunk + 1) // 2) + 16)

        @block.gpsimd
        def _(gp):
            for i, g in enumerate(chunks):
                s = i % 2
                gp.wait_ge(dsems[s], 32 * (i // 2 + 1))
                if i >= 2:
                    gp.wait_ge(mball_dve, i - 1)
                gp.tensor_scalar(objm[s][:, 0:g], lt[s][:, 0:g, 14:15, :],
                                 1.0, None, Alu.is_equal)
                gp.tensor_tensor(dclsp[s][:, 0:g], pt[s][:, 0:g, 24:30, :],
                                 lt[s][:, 0:g, 29:35, :], Alu.subtract)
                gp.drain()
                gp.wait_ge(u_done, i + 1)
                gp.tensor_scalar(rm[s][:, 0:g, 4:5, :], objm[s][:, 0:g],
                                 -0.70710678, 0.70710678, Alu.mult, Alu.add)
                gp.tensor_tensor(
                    rm[s][:, 0:g, 0:4, :],
                    objm[s][:, 0:g].unsqueeze(2).broadcast_to([P, g, 4, K]),
                    tq[s][:, 0:g], Alu.mult,
                ).then_inc(rm_done, 1)
                gp.drain()
                if i >= 2:
                    gp.wait_ge(acc_done, i - 1)
                gp.tensor_tensor(
                    mball[s][:, 0:g, 26:32, :], dclsp[s][:, 0:g],
                    objm[s][:, 0:g].unsqueeze(2).broadcast_to([P, g, 6, K]),
                    Alu.mult,
                )
                gp.tensor_tensor(
                    mball[s][:, 0:g, 10:12, :], pt[s][:, 0:g, 8:10, :],
                    rm[s][:, 0:g, 4:5, :].broadcast_to([P, g, 2, K]),
                    Alu.mult,
                )
                gp.tensor_tensor(
                    mball[s][:, 0:g, 8:10, :], dconf[s][:, 0:g],
                    rm[s][:, 0:g, 2:4, :], Alu.mult,
                ).then_inc(mball_pool, 1)

        @block.scalar
        def _(act):
            def sq_acc(j):
                sj = j % 2
                gj = chunks[j]
                act.wait_ge(mball_dve, j + 1)
                act.wait_ge(mball_pool, j + 1)
                act.activation(
                    junk32[sj][:, 0:gj].rearrange("p g c k -> p (g c k)"),
                    mball[sj][:, 0:gj].rearrange("p g c k -> p (g c k)"),
                    Act.Square,
                    accum_out=acc[:, j:j + 1],
                ).then_inc(acc_done, 1)

            for i, g in enumerate(chunks):
                s = i % 2
                act.wait_ge(dsems[s], 32 * (i // 2 + 1))
                act.activation(sqp[s][:, 0:g], pt[s][:, 0:g, 4:8, :], Act.Sqrt)
                act.activation(sql[s][:, 0:g], lt[s][:, 0:g, 10:14, :],
                               Act.Sqrt).then_inc(sqrt_done, 1)
                if i >= 2:
                    sq_acc(i - 2)
            sq_acc(nchunk - 2)
            sq_acc(nchunk - 1)

        @block.vector
        def _(v):
            tt = v.tensor_tensor
            ts = v.tensor_scalar

            def lagged_mults(j):
                sj = j % 2
                gj = chunks[j]
                v.wait_ge(rm_done, j + 1)
                if j >= 2:
                    v.wait_ge(acc_done, j - 1)
                tt(mball[sj][:, 0:gj, 0:4, :], dxyb[sj][:, 0:gj, 2:6, :],
                   rm[sj][:, 0:gj, 0:1, :].broadcast_to([P, gj, 4, K]),
                   Alu.mult)
                tt(mball[sj][:, 0:gj, 4:8, :], dxyb[sj][:, 0:gj, 6:10, :],
                   rm[sj][:, 0:gj, 1:2, :].broadcast_to([P, gj, 4, K]),
                   Alu.mult)
                tt(mball[sj][:, 0:gj, 12:26, :], dcls[sj][:, 0:gj],
                   objm[sj][:, 0:gj].unsqueeze(2).broadcast_to([P, gj, 14, K]),
                   Alu.mult).then_inc(mball_dve, 1)
                v.drain()

            for i, g in enumerate(chunks):
                s = i % 2
                p, l = pt[s], lt[s]
                v.wait_ge(dsems[s], 32 * (i // 2 + 1))
                # W1: reads only tiles
                tt(dxyb[s][:, 0:g, 2:4, :], p[:, 0:g, 0:2, :],
                   l[:, 0:g, 0:2, :], Alu.subtract)
                tt(dxyb[s][:, 0:g, 0:2, :], p[:, 0:g, 2:4, :],
                   l[:, 0:g, 2:4, :], Alu.subtract)
                tt(dxyb[s][:, 0:g, 6:8, :], p[:, 0:g, 2:4, :],
                   l[:, 0:g, 8:10, :], Alu.subtract)
                tt(swh[s][:, 0:g], p[:, 0:g, 4:8, :], l[:, 0:g, 4:8, :],
                   Alu.add)
                tt(mwh[s][:, 0:g], p[:, 0:g, 4:8, :], l[:, 0:g, 4:8, :],
                   Alu.min)
                tt(apw[s][:, 0:g],
                   p[:, 0:g, 4:8, :].rearrange("p g (b w) k -> p g b w k",
                                               b=2)[:, :, :, 0, :],
                   p[:, 0:g, 4:8, :].rearrange("p g (b w) k -> p g b w k",
                                               b=2)[:, :, :, 1, :],
                   Alu.mult)
                tt(agb[s][:, 0:g], l[:, 0:g, 4:5, :], l[:, 0:g, 5:6, :],
                   Alu.mult)
                tt(dcls[s][:, 0:g], p[:, 0:g, 10:24, :], l[:, 0:g, 15:29, :],
                   Alu.subtract)
                v.drain()
                # W2
                ts(adb[s][:, 0:g, 0:2, :].bitcast(U16),
                   dxyb[s][:, 0:g, 2:4, :].bitcast(U16),
                   0x7FFF, None, Alu.bitwise_and)
                ts(adb[s][:, 0:g, 2:4, :].bitcast(U16),
                   dxyb[s][:, 0:g, 0:2, :].bitcast(U16),
                   0x7FFF, None, Alu.bitwise_and)
                ts(s35[s][:, 0:g], swh[s][:, 0:g], 3.5, None, Alu.mult)
                ts(mwh7[s][:, 0:g], mwh[s][:, 0:g], 7.0, None, Alu.mult)
                tt(sa[s][:, 0:g], apw[s][:, 0:g],
                   agb[s][:, 0:g].broadcast_to([P, g, 2, K]), Alu.add)
                v.drain()
                # W3  (adb is box-reversed relative to s35: flip its view)
                tt(ta[s][:, 0:g], s35[s][:, 0:g],
                   adb[s][:, 0:g].rearrange("p g (a c) k -> p g a c k", a=2),
                   Alu.subtract)
                ts(sa49[s][:, 0:g], sa[s][:, 0:g], 49.0, None, Alu.mult)
                v.drain()
                tt(ov[s][:, 0:g], mwh7[s][:, 0:g], ta[s][:, 0:g], Alu.min)
                v.drain()
                ts(cl[s][:, 0:g], ov[s][:, 0:g], 0.0, None, Alu.max)
                v.drain()
                tt(ints[s][:, 0:g], cl[s][:, 0:g, :, 0:1, :],
                   cl[s][:, 0:g, :, 1:2, :], Alu.mult)
                v.drain()
                tt(u49[s][:, 0:g], sa49[s][:, 0:g], ints[s][:, 0:g],
                   Alu.subtract)
                v.drain()
                with nc.allow_low_precision(reason="bf16 iou tolerated"):
                    v.reciprocal(rcp[s][:, 0:g], u49[s][:, 0:g])
                v.drain()
                tt(iou[s][:, 0:g], ints[s][:, 0:g], rcp[s][:, 0:g], Alu.mult)
                v.drain()
                # W10: use1 + dconf
                tt(tq[s][:, 0:g, 0:1, :], iou[s][:, 0:g, 0:1, :],
                   iou[s][:, 0:g, 1:2, :], Alu.is_ge)
                tt(dconf[s][:, 0:g], p[:, 0:g, 8:10, :], iou[s][:, 0:g],
                   Alu.subtract)
                v.drain()
                # W10b: tq = linear sqrt-mask factors of u ; dsq
                ts(tq[s][:, 0:g, 1:2, :], tq[s][:, 0:g, 0:1, :],
                   -2.23606798, 2.23606798, Alu.mult, Alu.add)
                ts(tq[s][:, 0:g, 2:3, :], tq[s][:, 0:g, 0:1, :],
                   0.29289322, 0.70710678, Alu.mult, Alu.add)
                ts(tq[s][:, 0:g, 3:4, :], tq[s][:, 0:g, 0:1, :],
                   -0.29289322, 1.0, Alu.mult, Alu.add)
                ts(tq[s][:, 0:g, 0:1, :], tq[s][:, 0:g, 0:1, :],
                   2.23606798, None, Alu.mult)
                v.wait_ge(sqrt_done, i + 1)
                tt(dxyb[s][:, 0:g, 4:6, :], sqp[s][:, 0:g, 0:1, :, :],
                   sql[s][:, 0:g, 0:1, :, :], Alu.subtract)
                tt(dxyb[s][:, 0:g, 8:10, :], sqp[s][:, 0:g, 1:2, :, :],
                   sql[s][:, 0:g, 1:2, :, :],
                   Alu.subtract).then_inc(u_done, 1)
                v.drain().then_inc(tfree_dve, 1)
                if i >= 1:
                    lagged_mults(i - 1)
            lagged_mults(nchunk - 1)

    return nc


_NC_CACHE = {}


def _get_nc():
    if "nc" not in _NC_CACHE:
        _NC_CACHE["nc"] = build_nc()
    return _NC_CACHE["nc"]


def _to_bf16_repack(pred, labels):
    import ml_dtypes

    bf = ml_dtypes.bfloat16
    p = np.ascontiguousarray(pred, dtype=np.float32).reshape(B_TOTAL, C, K)
    l = np.ascontiguousarray(labels, dtype=np.float32).reshape(B_TOTAL, C, K)
    pb = p.astype(bf)
    lb = l.astype(bf)
    # obj channel: keep the ==1.0 test exact under rounding
    l4 = l[:, 4, :]
    lb4 = lb[:, 4, :]
    bad = (l4 != np.float32(1.0)) & (lb4.astype(np.float32) == np.float32(1.0))
    if bad.any():
        lb4[bad] = bf(0.99609375)
        lb[:, 4, :] = lb4
    prp = np.ascontiguousarray(pb[:, PP_IDX, :]).reshape(B_TOTAL, ROWP)
    lrp = np.ascontiguousarray(lb[:, LL_IDX, :]).reshape(B_TOTAL, ROWL)
    return prp, lrp


def run_device(pred, labels, trace=False):
    nc = _get_nc()
    prp, lrp = _to_bf16_repack(pred, labels)
    in_maps = []
    for c in range(NCORES):
        rows = slice(c * B_CORE, (c + 1) * B_CORE)
        in_maps.append({"pred": prp[rows], "labels": lrp[rows]})
    res = run_bass_kernel_spmd(nc, in_maps, list(range(NCORES)), trace=trace)
    total = 0.0
    for c in range(NCORES):
        total += float(res.results[c]["out"].astype(np.float64).sum())
    loss = np.float32(total / B_TOTAL)
    return loss, res


def kernel(pred, labels):
    loss, _ = run_device(pred, labels, trace=False)
    return np.array(loss, dtype=np.float32)


if __name__ == "__main__":
    rng = np.random.default_rng(0)
    p = rng.random((B_TOTAL, C, 7, 7), dtype=np.float32)
    l = rng.random((B_TOTAL, C, 7, 7), dtype=np.float32)
    l[:, 4] = (rng.random((B_TOTAL, 7, 7)) < 0.3).astype(np.float32)
    print(kernel(p, l))


# revision 25
# speedup vs baseline: 1.9669x; 1.0302x over previous
"""YOLOv1 loss kernel for Trainium2, 8-core data-parallel, bf16 pipeline.

Strategy: shard batch (8192) across 8 cores (1024 rows each). Host converts
inputs to bf16 (labels obj channel converted equality-preserving so l4 == 1.0
stays exact) and repacks channels per-row so every multi-channel device op is
one contiguous instruction:

  pred row (30 ch):  [0,1,5,6 | 2,3,7,8 | 4,9 | 10..29]
  label row (35 ch): [0,1,0,1 | 2,3,2,3 | 5,6 | 2,3,7,8 | 4 | 10..29]

Each core streams its shard in uneven chunks (g units of 128 rows, layout
[128, g, ch, 49]). Per chunk the whole loss reduces to ONE Scalar-engine
Square+accumulate over a packed masked buffer mball[g, 32, 49]:
  slots 0:8   sqrt(5*om_b) * (coor diffs: dx, dy, sqrt-w, sqrt-h per box)
  slots 8:10  sqrt(mA/mB) * (conf - iou) per box
  slots 10:12 sqrt(0.5*(1-obj)) * (p4, p9)
  slots 12:32 obj * (pred_cls - label_cls)
where om_u = obj*use1, om_nu = obj*(1-use1), mA = om_u + 0.5*om_nu,
mB = om_nu + 0.5*om_u. sum(mball^2) == chunk loss contribution exactly.

IoU runs in 7x-scaled units: ov7 = max(min(7*min(w), 3.5*(wp+wl) - |dc|), 0),
ints49 = ov7w*ov7h, u49 = 49*(area_p + area_g) - ints49, iou = ints49/u49.

Engine split: DVE does diffs/IoU (tensor_tensor 2x bf16, tensor_scalar 4x)
with the masked-multiply wave lagged one chunk behind (software pipelining);
Pool (gpsimd) builds the mask vector in 3 ops plus the small masked conf/q
multiplies; ACT does sqrts, one Sqrt(scale=5) mask-root op and the final
Square+accum. Out: acc[128, NCHUNK] fp32 per core, summed on host in fp64.
"""

import sys

import numpy as np

for _p in ("/opt/trn_rl_repo", "/root/.axon_site/_ro/trn_rl_repo"):
    if _p not in sys.path:
        sys.path.insert(0, _p)

import concourse.bass as bass
import concourse.mybir as mybir
from concourse.bass_utils import run_bass_kernel_spmd

F32 = mybir.dt.float32
BF16 = mybir.dt.bfloat16
U16 = mybir.dt.uint16
Alu = mybir.AluOpType
Act = mybir.ActivationFunctionType

B_TOTAL = 8192
NCORES = 8
B_CORE = B_TOTAL // NCORES  # 1024
P = 128
C = 30
K = 49
CP = 30   # repacked pred channels
CL = 35   # repacked label channels
ROWP = CP * K
ROWL = CL * K

# host channel permutations
PP_IDX = [0, 1, 5, 6, 2, 3, 7, 8, 4, 9] + list(range(10, 30))
LL_IDX = [0, 1, 0, 1, 2, 3, 2, 3, 5, 6, 2, 3, 7, 8, 4] + list(range(10, 30))

CHUNKS = (1, 3, 3, 1)


def build_nc(chunks=CHUNKS):
    assert sum(chunks) * P == B_CORE
    nchunk = len(chunks)
    maxg = max(chunks)
    nc = bass.Bass()
    pred = nc.declare_dram_parameter("pred", [B_CORE, ROWP], BF16, isOutput=False)
    labels = nc.declare_dram_parameter("labels", [B_CORE, ROWL], BF16,
                                       isOutput=False)
    out = nc.declare_dram_parameter("out", [P, nchunk], F32, isOutput=True)

    from contextlib import ExitStack

    _ctr = [0]
    es = ExitStack()

    def sb(shape, dt=BF16):
        _ctr[0] += 1
        return es.enter_context(nc.sbuf_tensor(f"t{_ctr[0]}", shape, dt))

    with es:
        pt = [sb([P, maxg, CP, K]) for _ in range(2)]
        lt = [sb([P, maxg, CL, K]) for _ in range(2)]
        # dxyb slots: 0:2 b2-iou-xy, 2:4 b1-xy, 4:6 b1-sqrt, 6:8 b2-xy, 8:10 b2-sqrt
        dxyb = [sb([P, maxg, 10, K]) for _ in range(2)]
        adb = [sb([P, maxg, 4, K]) for _ in range(2)]     # |b2Ix,b2Iy,b1x,b1y|
        swh = [sb([P, maxg, 2, 2, K]) for _ in range(2)]
        s35 = [sb([P, maxg, 2, 2, K]) for _ in range(2)]
        mwh = [sb([P, maxg, 2, 2, K]) for _ in range(2)]
        mwh7 = [sb([P, maxg, 2, 2, K]) for _ in range(2)]
        ta = [sb([P, maxg, 2, 2, K]) for _ in range(2)]
        ov = [sb([P, maxg, 2, 2, K]) for _ in range(2)]
        cl = [sb([P, maxg, 2, 2, K]) for _ in range(2)]
        ints = [sb([P, maxg, 2, K]) for _ in range(2)]
        apw = [sb([P, maxg, 2, K]) for _ in range(2)]
        agb = [sb([P, maxg, 1, K]) for _ in range(2)]
        sa = [sb([P, maxg, 2, K]) for _ in range(2)]
        sa49 = [sb([P, maxg, 2, K]) for _ in range(2)]
        u49 = [sb([P, maxg, 2, K]) for _ in range(2)]
        rcp = [sb([P, maxg, 2, K]) for _ in range(2)]
        iou = [sb([P, maxg, 2, K]) for _ in range(2)]
        tq = [sb([P, maxg, 4, K]) for _ in range(2)]      # use1, t3, t1, t2
        dconf = [sb([P, maxg, 2, K]) for _ in range(2)]
        dcls = [sb([P, maxg, 14, K]) for _ in range(2)]
        dclsp = [sb([P, maxg, 6, K]) for _ in range(2)]
        sqp = [sb([P, maxg, 2, 2, K]) for _ in range(2)]
        sql = [sb([P, maxg, 2, 2, K]) for _ in range(2)]
        objm = [sb([P, maxg, K]) for _ in range(2)]
        rm = [sb([P, maxg, 5, K]) for _ in range(2)]
        mball = [sb([P, maxg, 32, K]) for _ in range(2)]
        junk32 = [sb([P, maxg, 32, K]) for _ in range(2)]
        acc = es.enter_context(nc.sbuf_tensor("acc", [P, nchunk], F32))

        dsemA = es.enter_context(nc.semaphore("dsemA"))
        dsemB = es.enter_context(nc.semaphore("dsemB"))
        dsems = [dsemA, dsemB]
        u_done = es.enter_context(nc.semaphore("u_done"))
        sqrt_done = es.enter_context(nc.semaphore("sqrt_done"))
        rm_done = es.enter_context(nc.semaphore("rm_done"))
        mball_dve = es.enter_context(nc.semaphore("mball_dve"))
        mball_pool = es.enter_context(nc.semaphore("mball_pool"))
        acc_done = es.enter_context(nc.semaphore("acc_done"))
        tfree_dve = es.enter_context(nc.semaphore("tfree_dve"))
        area_done = es.enter_context(nc.semaphore("area_done"))
        dsemT = es.enter_context(nc.semaphore("dsemT"))
        block = es.enter_context(nc.Block())

        offs = [0]
        for g in chunks:
            offs.append(offs[-1] + g * P)

        # dsem thresholds: chunk 0 is split into head (iou ch) + tail (cls ch)
        head_v = {}
        _dv = [0, 0]
        for i in range(nchunk):
            s = i % 2
            head_v[i] = _dv[s] + 32
            _dv[s] += 32

        @block.sync
        def _(sync):
            for i, g in enumerate(chunks):
                s = i % 2
                if i >= 2:
                    sync.wait_ge(sqrt_done, i - 1)
                    sync.wait_ge(mball_pool, i - 1)
                    sync.wait_ge(tfree_dve, i - 1)
                rows = slice(offs[i], offs[i + 1])
                if i == 0:
                    sync.dma_start(
                        out=pt[s][:, 0:g, 0:10, :].rearrange(
                            "p g c k -> p g (c k)"),
                        in_=pred[rows, 0:10 * K].rearrange(
                            "(g p) d -> p g d", p=P),
                    ).then_inc(dsems[s], 16)
                    sync.dma_start(
                        out=lt[s][:, 0:g, 0:15, :].rearrange(
                            "p g c k -> p g (c k)"),
                        in_=labels[rows, 0:15 * K].rearrange(
                            "(g p) d -> p g d", p=P),
                    ).then_inc(dsems[s], 16)
                    sync.dma_start(
                        out=pt[s][:, 0:g, 10:30, :].rearrange(
                            "p g c k -> p g (c k)"),
                        in_=pred[rows, 10 * K:].rearrange(
                            "(g p) d -> p g d", p=P),
                    ).then_inc(dsemT, 16)
                    sync.dma_start(
                        out=lt[s][:, 0:g, 15:35, :].rearrange(
                            "p g c k -> p g (c k)"),
                        in_=labels[rows, 15 * K:].rearrange(
                            "(g p) d -> p g d", p=P),
                    ).then_inc(dsemT, 16)
                else:
                    sync.dma_start(
                        out=pt[s][:, 0:g].rearrange("p g c k -> p g (c k)"),
                        in_=pred[rows].rearrange("(g p) d -> p g d", p=P),
                    ).then_inc(dsems[s], 16)
                    sync.dma_start(
                        out=lt[s][:, 0:g].rearrange("p g c k -> p g (c k)"),
                        in_=labels[rows].rearrange("(g p) d -> p g d", p=P),
                    ).then_inc(dsems[s], 16)
            sync.wait_ge(acc_done, nchunk)
            sync.dma_start(out=out[:], in_=acc[:]).then_inc(dsemA, 16)
            sync.wait_ge(dsemA, _dv[0] + 16)

        @block.gpsimd
        def _(gp):
            for i, g in enumerate(chunks):
                s = i % 2
                gp.wait_ge(dsems[s], head_v[i])
                if i >= 2:
                    gp.wait_ge(mball_dve, i - 1)
                gp.tensor_scalar(objm[s][:, 0:g], lt[s][:, 0:g, 14:15, :],
                                 1.0, None, Alu.is_equal)
                if i == 0:
                    gp.wait_ge(dsemT, 32)
                gp.tensor_tensor(dclsp[s][:, 0:g], pt[s][:, 0:g, 24:30, :],
                                 lt[s][:, 0:g, 29:35, :], Alu.subtract)
                gp.drain()
                gp.wait_ge(u_done, i + 1)
                gp.tensor_scalar(rm[s][:, 0:g, 4:5, :], objm[s][:, 0:g],
                                 -0.70710678, 0.70710678, Alu.mult, Alu.add)
                gp.tensor_tensor(
                    rm[s][:, 0:g, 0:4, :],
                    objm[s][:, 0:g].unsqueeze(2).broadcast_to([P, g, 4, K]),
                    tq[s][:, 0:g], Alu.mult,
                ).then_inc(rm_done, 1)
                gp.drain()
                if i >= 2:
                    gp.wait_ge(acc_done, i - 1)
                gp.tensor_tensor(
                    mball[s][:, 0:g, 26:32, :], dclsp[s][:, 0:g],
                    objm[s][:, 0:g].unsqueeze(2).broadcast_to([P, g, 6, K]),
                    Alu.mult,
                )
                gp.tensor_tensor(
                    mball[s][:, 0:g, 10:12, :], pt[s][:, 0:g, 8:10, :],
                    rm[s][:, 0:g, 4:5, :].broadcast_to([P, g, 2, K]),
                    Alu.mult,
                )
                gp.tensor_tensor(
                    mball[s][:, 0:g, 8:10, :], dconf[s][:, 0:g],
                    rm[s][:, 0:g, 2:4, :], Alu.mult,
                ).then_inc(mball_pool, 1)

        @block.scalar
        def _(act):
            def sq_acc(j):
                sj = j % 2
                gj = chunks[j]
                act.wait_ge(mball_dve, j + 1)
                act.wait_ge(mball_pool, j + 1)
                act.activation(
                    junk32[sj][:, 0:gj].rearrange("p g c k -> p (g c k)"),
                    mball[sj][:, 0:gj].rearrange("p g c k -> p (g c k)"),
                    Act.Square,
                    accum_out=acc[:, j:j + 1],
                ).then_inc(acc_done, 1)

            for i, g in enumerate(chunks):
                s = i % 2
                act.wait_ge(dsems[s], head_v[i])
                act.activation(sqp[s][:, 0:g], pt[s][:, 0:g, 4:8, :], Act.Sqrt)
                act.activation(sql[s][:, 0:g], lt[s][:, 0:g, 10:14, :],
                               Act.Sqrt).then_inc(sqrt_done, 1)
                if i >= 2:
                    sq_acc(i - 2)
            sq_acc(nchunk - 2)
            sq_acc(nchunk - 1)

        @block.vector
        def _(v):
            tt = v.tensor_tensor
            ts = v.tensor_scalar

            def lagged_mults(j):
                sj = j % 2
                gj = chunks[j]
                v.wait_ge(rm_done, j + 1)
                if j >= 2:
                    v.wait_ge(acc_done, j - 1)
                tt(mball[sj][:, 0:gj, 0:4, :], dxyb[sj][:, 0:gj, 2:6, :],
                   rm[sj][:, 0:gj, 0:1, :].broadcast_to([P, gj, 4, K]),
                   Alu.mult)
                tt(mball[sj][:, 0:gj, 4:8, :], dxyb[sj][:, 0:gj, 6:10, :],
                   rm[sj][:, 0:gj, 1:2, :].broadcast_to([P, gj, 4, K]),
                   Alu.mult)
                tt(mball[sj][:, 0:gj, 12:26, :], dcls[sj][:, 0:gj],
                   objm[sj][:, 0:gj].unsqueeze(2).broadcast_to([P, gj, 14, K]),
                   Alu.mult).then_inc(mball_dve, 1)
                v.drain()

            for i, g in enumerate(chunks):
                s = i % 2
                p, l = pt[s], lt[s]
                if i == nchunk - 1 and i >= 1:
                    lagged_mults(i - 1)
                v.wait_ge(dsems[s], head_v[i])
                # W1: reads only tiles
                tt(dxyb[s][:, 0:g, 2:4, :], p[:, 0:g, 0:2, :],
                   l[:, 0:g, 0:2, :], Alu.subtract)
                tt(dxyb[s][:, 0:g, 0:2, :], p[:, 0:g, 2:4, :],
                   l[:, 0:g, 2:4, :], Alu.subtract)
                tt(dxyb[s][:, 0:g, 6:8, :], p[:, 0:g, 2:4, :],
                   l[:, 0:g, 8:10, :], Alu.subtract)
                tt(swh[s][:, 0:g], p[:, 0:g, 4:8, :], l[:, 0:g, 4:8, :],
                   Alu.add)
                tt(mwh[s][:, 0:g], p[:, 0:g, 4:8, :], l[:, 0:g, 4:8, :],
                   Alu.min)
                tt(apw[s][:, 0:g],
                   p[:, 0:g, 4:8, :].rearrange("p g (b w) k -> p g b w k",
                                               b=2)[:, :, :, 0, :],
                   p[:, 0:g, 4:8, :].rearrange("p g (b w) k -> p g b w k",
                                               b=2)[:, :, :, 1, :],
                   Alu.mult)
                tt(agb[s][:, 0:g], l[:, 0:g, 4:5, :], l[:, 0:g, 5:6, :],
                   Alu.mult)
                if i == 0:
                    v.wait_ge(dsemT, 32)
                tt(dcls[s][:, 0:g], p[:, 0:g, 10:24, :], l[:, 0:g, 15:29, :],
                   Alu.subtract)
                v.drain()
                # W2
                ts(adb[s][:, 0:g, 0:2, :].bitcast(U16),
                   dxyb[s][:, 0:g, 2:4, :].bitcast(U16),
                   0x7FFF, None, Alu.bitwise_and)
                ts(adb[s][:, 0:g, 2:4, :].bitcast(U16),
                   dxyb[s][:, 0:g, 0:2, :].bitcast(U16),
                   0x7FFF, None, Alu.bitwise_and)
                ts(s35[s][:, 0:g], swh[s][:, 0:g], 3.5, None, Alu.mult)
                ts(mwh7[s][:, 0:g], mwh[s][:, 0:g], 7.0, None, Alu.mult)
                tt(sa[s][:, 0:g], apw[s][:, 0:g],
                   agb[s][:, 0:g].broadcast_to([P, g, 2, K]), Alu.add)
                v.drain()
                # W3  (adb is box-reversed relative to s35: flip its view)
                tt(ta[s][:, 0:g], s35[s][:, 0:g],
                   adb[s][:, 0:g].rearrange("p g (a c) k -> p g a c k", a=2),
                   Alu.subtract)
                ts(sa49[s][:, 0:g], sa[s][:, 0:g], 49.0, None, Alu.mult)
                v.drain()
                tt(ov[s][:, 0:g], mwh7[s][:, 0:g], ta[s][:, 0:g], Alu.min)
                v.drain()
                ts(cl[s][:, 0:g], ov[s][:, 0:g], 0.0, None, Alu.max)
                v.drain()
                tt(ints[s][:, 0:g], cl[s][:, 0:g, :, 0:1, :],
                   cl[s][:, 0:g, :, 1:2, :], Alu.mult)
                v.drain()
                tt(u49[s][:, 0:g], sa49[s][:, 0:g], ints[s][:, 0:g],
                   Alu.subtract)
                v.drain()
                with nc.allow_low_precision(reason="bf16 iou tolerated"):
                    v.reciprocal(rcp[s][:, 0:g], u49[s][:, 0:g])
                v.drain()
                tt(iou[s][:, 0:g], ints[s][:, 0:g], rcp[s][:, 0:g], Alu.mult)
                v.drain()
                # W10: use1 + dconf
                tt(tq[s][:, 0:g, 0:1, :], iou[s][:, 0:g, 0:1, :],
                   iou[s][:, 0:g, 1:2, :], Alu.is_ge)
                tt(dconf[s][:, 0:g], p[:, 0:g, 8:10, :], iou[s][:, 0:g],
                   Alu.subtract)
                v.drain()
                # W10b: tq = linear sqrt-mask factors of u ; dsq
                ts(tq[s][:, 0:g, 1:2, :], tq[s][:, 0:g, 0:1, :],
                   -2.23606798, 2.23606798, Alu.mult, Alu.add)
                ts(tq[s][:, 0:g, 2:3, :], tq[s][:, 0:g, 0:1, :],
                   0.29289322, 0.70710678, Alu.mult, Alu.add)
                ts(tq[s][:, 0:g, 3:4, :], tq[s][:, 0:g, 0:1, :],
                   -0.29289322, 1.0, Alu.mult, Alu.add)
                ts(tq[s][:, 0:g, 0:1, :], tq[s][:, 0:g, 0:1, :],
                   2.23606798, None, Alu.mult)
                v.wait_ge(sqrt_done, i + 1)
                tt(dxyb[s][:, 0:g, 4:6, :], sqp[s][:, 0:g, 0:1, :, :],
                   sql[s][:, 0:g, 0:1, :, :], Alu.subtract)
                tt(dxyb[s][:, 0:g, 8:10, :], sqp[s][:, 0:g, 1:2, :, :],
                   sql[s][:, 0:g, 1:2, :, :],
                   Alu.subtract).then_inc(u_done, 1)
                v.drain().then_inc(tfree_dve, 1)
                if i >= 1 and i != nchunk - 1:
                    lagged_mults(i - 1)
            lagged_mults(nchunk - 1)

    return nc


_NC_CACHE = {}


def _get_nc():
    if "nc" not in _NC_CACHE:
        _NC_CACHE["nc"] = build_nc()
    return _NC_CACHE["nc"]


def _to_bf16_repack(pred, labels):
    import ml_dtypes

    bf = ml_dtypes.bfloat16
    p = np.ascontiguousarray(pred, dtype=np.float32).reshape(B_TOTAL, C, K)
    l = np.ascontiguousarray(labels, dtype=np.float32).reshape(B_TOTAL, C, K)
    pb = p.astype(bf)
    lb = l.astype(bf)
    # obj channel: keep the ==1.0 test exact under rounding
    l4 = l[:, 4, :]
    lb4 = lb[:, 4, :]
    bad = (l4 != np.float32(1.0)) & (lb4.astype(np.float32) == np.float32(1.0))
    if bad.any():
        lb4[bad] = bf(0.99609375)
        lb[:, 4, :] = lb4
    prp = np.ascontiguousarray(pb[:, PP_IDX, :]).reshape(B_TOTAL, ROWP)
    lrp = np.ascontiguousarray(lb[:, LL_IDX, :]).reshape(B_TOTAL, ROWL)
    return prp, lrp


def run_device(pred, labels, trace=False):
    nc = _get_nc()
    prp, lrp = _to_bf16_repack(pred, labels)
    in_maps = []
    for c in range(NCORES):
        rows = slice(c * B_CORE, (c + 1) * B_CORE)
        in_maps.append({"pred": prp[rows], "labels": lrp[rows]})
    res = run_bass_kernel_spmd(nc, in_maps, list(range(NCORES)), trace=trace)
    total = 0.0
    for c in range(NCORES):
        total += float(res.results[c]["out"].astype(np.float64).sum())
    loss = np.float32(total / B_TOTAL)
    return loss, res


def kernel(pred, labels):
    loss, _ = run_device(pred, labels, trace=False)
    return np.array(loss, dtype=np.float32)


if __name__ == "__main__":
    rng = np.random.default_rng(0)
    p = rng.random((B_TOTAL, C, 7, 7), dtype=np.float32)
    l = rng.random((B_TOTAL, C, 7, 7), dtype=np.float32)
    l[:, 4] = (rng.random((B_TOTAL, 7, 7)) < 0.3).astype(np.float32)
    print(kernel(p, l))


# revision 29
# speedup vs baseline: 1.9767x; 1.0050x over previous
"""YOLOv1 loss kernel for Trainium2, 8-core data-parallel, bf16 pipeline.

Strategy: shard batch (8192) across 8 cores (1024 rows each). Host converts
inputs to bf16 (labels obj channel converted equality-preserving so l4 == 1.0
stays exact) and repacks channels per-row so every multi-channel device op is
one contiguous instruction:

  pred row (30 ch):  [0,1,5,6 | 2,3,7,8 | 4,9 | 10..29]
  label row (35 ch): [0,1,0,1 | 2,3,2,3 | 5,6 | 2,3,7,8 | 4 | 10..29]

Each core streams its shard in uneven chunks (g units of 128 rows, layout
[128, g, ch, 49]). Per chunk the whole loss reduces to ONE Scalar-engine
Square+accumulate over a packed masked buffer mball[g, 32, 49]:
  slots 0:8   sqrt(5*om_b) * (coor diffs: dx, dy, sqrt-w, sqrt-h per box)
  slots 8:10  sqrt(mA/mB) * (conf - iou) per box
  slots 10:12 sqrt(0.5*(1-obj)) * (p4, p9)
  slots 12:32 obj * (pred_cls - label_cls)
where om_u = obj*use1, om_nu = obj*(1-use1), mA = om_u + 0.5*om_nu,
mB = om_nu + 0.5*om_u. sum(mball^2) == chunk loss contribution exactly.

IoU runs in 7x-scaled units: ov7 = max(min(7*min(w), 3.5*(wp+wl) - |dc|), 0),
ints49 = ov7w*ov7h, u49 = 49*(area_p + area_g) - ints49, iou = ints49/u49.

All sqrt-mask values are linear in use1 (e.g. sqrt(mA) = 0.7071 + 0.2929*u,
sqrt(5*om_u) = sqrt(5)*obj*u), so the mask vector rm[5] is built with plain
mult/add ops - no activation-engine involvement on the mask path.

Engine split: DVE does diffs/IoU (tensor_tensor 2x bf16, tensor_scalar 4x)
with the masked-multiply wave lagged one chunk behind (software pipelining);
Pool (gpsimd) computes obj, the rm vector, 8 of 20 cls channels and the
masked conf/q multiplies; ACT does the w/h sqrts and one Square+accum_out
per chunk (lagged two chunks so sqrts never queue behind it). Chunk 0's DMA
is split into an IoU-channel head and cls tail so compute starts early.
Out: acc[128, NCHUNK] fp32 per core, summed on host in fp64.
"""

import sys

import numpy as np

for _p in ("/opt/trn_rl_repo", "/root/.axon_site/_ro/trn_rl_repo"):
    if _p not in sys.path:
        sys.path.insert(0, _p)

import concourse.bass as bass
import concourse.mybir as mybir
from concourse.bass_utils import run_bass_kernel_spmd

F32 = mybir.dt.float32
BF16 = mybir.dt.bfloat16
U16 = mybir.dt.uint16
Alu = mybir.AluOpType
Act = mybir.ActivationFunctionType

B_TOTAL = 8192
NCORES = 8
B_CORE = B_TOTAL // NCORES  # 1024
P = 128
C = 30
K = 49
CP = 30   # repacked pred channels
CL = 35   # repacked label channels
ROWP = CP * K
ROWL = CL * K

# host channel permutations
PP_IDX = [0, 1, 5, 6, 2, 3, 7, 8, 4, 9] + list(range(10, 30))
LL_IDX = [0, 1, 0, 1, 2, 3, 2, 3, 5, 6, 2, 3, 7, 8, 4] + list(range(10, 30))

CHUNKS = (1, 3, 3, 1)


def build_nc(chunks=CHUNKS):
    assert sum(chunks) * P == B_CORE
    nchunk = len(chunks)
    maxg = max(chunks)
    nc = bass.Bass()
    pred = nc.declare_dram_parameter("pred", [B_CORE, ROWP], BF16, isOutput=False)
    labels = nc.declare_dram_parameter("labels", [B_CORE, ROWL], BF16,
                                       isOutput=False)
    out = nc.declare_dram_parameter("out", [P, nchunk], F32, isOutput=True)

    from contextlib import ExitStack

    _ctr = [0]
    es = ExitStack()

    def sb(shape, dt=BF16):
        _ctr[0] += 1
        return es.enter_context(nc.sbuf_tensor(f"t{_ctr[0]}", shape, dt))

    with es:
        pt = [sb([P, maxg, CP, K]) for _ in range(2)]
        lt = [sb([P, maxg, CL, K]) for _ in range(2)]
        # dxyb slots: 0:2 b2-iou-xy, 2:4 b1-xy, 4:6 b1-sqrt, 6:8 b2-xy, 8:10 b2-sqrt
        dxyb = [sb([P, maxg, 10, K]) for _ in range(2)]
        adb = [sb([P, maxg, 4, K]) for _ in range(2)]     # |b2Ix,b2Iy,b1x,b1y|
        swh = [sb([P, maxg, 2, 2, K]) for _ in range(2)]
        s35 = [sb([P, maxg, 2, 2, K]) for _ in range(2)]
        mwh = [sb([P, maxg, 2, 2, K]) for _ in range(2)]
        mwh7 = [sb([P, maxg, 2, 2, K]) for _ in range(2)]
        ta = [sb([P, maxg, 2, 2, K]) for _ in range(2)]
        ov = [sb([P, maxg, 2, 2, K]) for _ in range(2)]
        cl = [sb([P, maxg, 2, 2, K]) for _ in range(2)]
        ints = [sb([P, maxg, 2, K]) for _ in range(2)]
        apw = [sb([P, maxg, 2, K]) for _ in range(2)]
        agb = [sb([P, maxg, 1, K]) for _ in range(2)]
        sa = [sb([P, maxg, 2, K]) for _ in range(2)]
        sa49 = [sb([P, maxg, 2, K]) for _ in range(2)]
        u49 = [sb([P, maxg, 2, K]) for _ in range(2)]
        rcp = [sb([P, maxg, 2, K]) for _ in range(2)]
        iou = [sb([P, maxg, 2, K]) for _ in range(2)]
        tq = [sb([P, maxg, 4, K]) for _ in range(2)]      # use1, t3, t1, t2
        dconf = [sb([P, maxg, 2, K]) for _ in range(2)]
        dcls = [sb([P, maxg, 12, K]) for _ in range(2)]
        dclsp = [sb([P, maxg, 8, K]) for _ in range(2)]
        sqp = [sb([P, maxg, 2, 2, K]) for _ in range(2)]
        sql = [sb([P, maxg, 2, 2, K]) for _ in range(2)]
        objm = [sb([P, maxg, K]) for _ in range(2)]
        rm = [sb([P, maxg, 5, K]) for _ in range(2)]
        mball = [sb([P, maxg, 32, K]) for _ in range(2)]
        junk32 = [sb([P, maxg, 32, K]) for _ in range(2)]
        acc = es.enter_context(nc.sbuf_tensor("acc", [P, nchunk], F32))

        dsemA = es.enter_context(nc.semaphore("dsemA"))
        dsemB = es.enter_context(nc.semaphore("dsemB"))
        dsems = [dsemA, dsemB]
        u_done = es.enter_context(nc.semaphore("u_done"))
        sqrt_done = es.enter_context(nc.semaphore("sqrt_done"))
        rm_done = es.enter_context(nc.semaphore("rm_done"))
        mball_dve = es.enter_context(nc.semaphore("mball_dve"))
        mball_pool = es.enter_context(nc.semaphore("mball_pool"))
        acc_done = es.enter_context(nc.semaphore("acc_done"))
        tfree_dve = es.enter_context(nc.semaphore("tfree_dve"))
        area_done = es.enter_context(nc.semaphore("area_done"))
        dsemT = es.enter_context(nc.semaphore("dsemT"))
        block = es.enter_context(nc.Block())

        offs = [0]
        for g in chunks:
            offs.append(offs[-1] + g * P)

        # dsem thresholds: chunk 0 is split into head (iou ch) + tail (cls ch)
        head_v = {}
        _dv = [0, 0]
        for i in range(nchunk):
            s = i % 2
            head_v[i] = _dv[s] + 32
            _dv[s] += 32

        @block.sync
        def _(sync):
            for i, g in enumerate(chunks):
                s = i % 2
                if i >= 2:
                    sync.wait_ge(sqrt_done, i - 1)
                    sync.wait_ge(mball_pool, i - 1)
                    sync.wait_ge(tfree_dve, i - 1)
                rows = slice(offs[i], offs[i + 1])
                if i == 0:
                    sync.dma_start(
                        out=pt[s][:, 0:g, 0:10, :].rearrange(
                            "p g c k -> p g (c k)"),
                        in_=pred[rows, 0:10 * K].rearrange(
                            "(g p) d -> p g d", p=P),
                    ).then_inc(dsems[s], 16)
                    sync.dma_start(
                        out=lt[s][:, 0:g, 0:15, :].rearrange(
                            "p g c k -> p g (c k)"),
                        in_=labels[rows, 0:15 * K].rearrange(
                            "(g p) d -> p g d", p=P),
                    ).then_inc(dsems[s], 16)
                    sync.dma_start(
                        out=pt[s][:, 0:g, 10:30, :].rearrange(
                            "p g c k -> p g (c k)"),
                        in_=pred[rows, 10 * K:].rearrange(
                            "(g p) d -> p g d", p=P),
                    ).then_inc(dsemT, 16)
                    sync.dma_start(
                        out=lt[s][:, 0:g, 15:35, :].rearrange(
                            "p g c k -> p g (c k)"),
                        in_=labels[rows, 15 * K:].rearrange(
                            "(g p) d -> p g d", p=P),
                    ).then_inc(dsemT, 16)
                else:
                    sync.dma_start(
                        out=pt[s][:, 0:g].rearrange("p g c k -> p g (c k)"),
                        in_=pred[rows].rearrange("(g p) d -> p g d", p=P),
                    ).then_inc(dsems[s], 16)
                    sync.dma_start(
                        out=lt[s][:, 0:g].rearrange("p g c k -> p g (c k)"),
                        in_=labels[rows].rearrange("(g p) d -> p g d", p=P),
                    ).then_inc(dsems[s], 16)
            sync.wait_ge(acc_done, nchunk)
            sync.dma_start(out=out[:], in_=acc[:]).then_inc(dsemA, 16)
            sync.wait_ge(dsemA, _dv[0] + 16)

        @block.gpsimd
        def _(gp):
            for i, g in enumerate(chunks):
                s = i % 2
                gp.wait_ge(dsems[s], head_v[i])
                if i >= 2:
                    gp.wait_ge(mball_dve, i - 1)
                gp.tensor_scalar(objm[s][:, 0:g], lt[s][:, 0:g, 14:15, :],
                                 1.0, None, Alu.is_equal)
                if i == 0:
                    gp.wait_ge(dsemT, 32)
                gp.tensor_tensor(dclsp[s][:, 0:g], pt[s][:, 0:g, 22:30, :],
                                 lt[s][:, 0:g, 27:35, :], Alu.subtract)
                gp.drain()
                gp.wait_ge(u_done, i + 1)
                gp.tensor_scalar(rm[s][:, 0:g, 4:5, :], objm[s][:, 0:g],
                                 -0.70710678, 0.70710678, Alu.mult, Alu.add)
                gp.tensor_tensor(
                    rm[s][:, 0:g, 0:4, :],
                    objm[s][:, 0:g].unsqueeze(2).broadcast_to([P, g, 4, K]),
                    tq[s][:, 0:g], Alu.mult,
                ).then_inc(rm_done, 1)
                gp.drain()
                if i >= 2:
                    gp.wait_ge(acc_done, i - 1)
                gp.tensor_tensor(
                    mball[s][:, 0:g, 24:32, :], dclsp[s][:, 0:g],
                    objm[s][:, 0:g].unsqueeze(2).broadcast_to([P, g, 8, K]),
                    Alu.mult,
                )
                gp.tensor_tensor(
                    mball[s][:, 0:g, 10:12, :], pt[s][:, 0:g, 8:10, :],
                    rm[s][:, 0:g, 4:5, :].broadcast_to([P, g, 2, K]),
                    Alu.mult,
                )
                gp.tensor_tensor(
                    mball[s][:, 0:g, 8:10, :], dconf[s][:, 0:g],
                    rm[s][:, 0:g, 2:4, :], Alu.mult,
                ).then_inc(mball_pool, 1)

        @block.scalar
        def _(act):
            def sq_acc(j):
                sj = j % 2
                gj = chunks[j]
                act.wait_ge(mball_dve, j + 1)
                act.wait_ge(mball_pool, j + 1)
                act.activation(
                    junk32[sj][:, 0:gj].rearrange("p g c k -> p (g c k)"),
                    mball[sj][:, 0:gj].rearrange("p g c k -> p (g c k)"),
                    Act.Square,
                    accum_out=acc[:, j:j + 1],
                ).then_inc(acc_done, 1)

            for i, g in enumerate(chunks):
                s = i % 2
                act.wait_ge(dsems[s], head_v[i])
                act.activation(sqp[s][:, 0:g], pt[s][:, 0:g, 4:8, :], Act.Sqrt)
                act.activation(sql[s][:, 0:g], lt[s][:, 0:g, 10:14, :],
                               Act.Sqrt).then_inc(sqrt_done, 1)
                if i >= 2:
                    sq_acc(i - 2)
            sq_acc(nchunk - 2)
            sq_acc(nchunk - 1)

        @block.vector
        def _(v):
            tt = v.tensor_tensor
            ts = v.tensor_scalar

            def lagged_mults(j):
                sj = j % 2
                gj = chunks[j]
                v.wait_ge(rm_done, j + 1)
                if j >= 2:
                    v.wait_ge(acc_done, j - 1)
                tt(mball[sj][:, 0:gj, 0:4, :], dxyb[sj][:, 0:gj, 2:6, :],
                   rm[sj][:, 0:gj, 0:1, :].broadcast_to([P, gj, 4, K]),
                   Alu.mult)
                tt(mball[sj][:, 0:gj, 4:8, :], dxyb[sj][:, 0:gj, 6:10, :],
                   rm[sj][:, 0:gj, 1:2, :].broadcast_to([P, gj, 4, K]),
                   Alu.mult)
                tt(mball[sj][:, 0:gj, 12:24, :], dcls[sj][:, 0:gj],
                   objm[sj][:, 0:gj].unsqueeze(2).broadcast_to([P, gj, 12, K]),
                   Alu.mult).then_inc(mball_dve, 1)
                v.drain()

            for i, g in enumerate(chunks):
                s = i % 2
                p, l = pt[s], lt[s]
                if i == nchunk - 1 and i >= 1:
                    lagged_mults(i - 1)
                v.wait_ge(dsems[s], head_v[i])
                # W1: reads only tiles
                tt(dxyb[s][:, 0:g, 2:4, :], p[:, 0:g, 0:2, :],
                   l[:, 0:g, 0:2, :], Alu.subtract)
                tt(dxyb[s][:, 0:g, 0:2, :], p[:, 0:g, 2:4, :],
                   l[:, 0:g, 2:4, :], Alu.subtract)
                tt(dxyb[s][:, 0:g, 6:8, :], p[:, 0:g, 2:4, :],
                   l[:, 0:g, 8:10, :], Alu.subtract)
                tt(swh[s][:, 0:g], p[:, 0:g, 4:8, :], l[:, 0:g, 4:8, :],
                   Alu.add)
                tt(mwh[s][:, 0:g], p[:, 0:g, 4:8, :], l[:, 0:g, 4:8, :],
                   Alu.min)
                tt(apw[s][:, 0:g],
                   p[:, 0:g, 4:8, :].rearrange("p g (b w) k -> p g b w k",
                                               b=2)[:, :, :, 0, :],
                   p[:, 0:g, 4:8, :].rearrange("p g (b w) k -> p g b w k",
                                               b=2)[:, :, :, 1, :],
                   Alu.mult)
                tt(agb[s][:, 0:g], l[:, 0:g, 4:5, :], l[:, 0:g, 5:6, :],
                   Alu.mult)
                if i == 0:
                    v.wait_ge(dsemT, 32)
                tt(dcls[s][:, 0:g], p[:, 0:g, 10:22, :], l[:, 0:g, 15:27, :],
                   Alu.subtract)
                v.drain()
                # W2
                ts(adb[s][:, 0:g, 0:2, :].bitcast(U16),
                   dxyb[s][:, 0:g, 2:4, :].bitcast(U16),
                   0x7FFF, None, Alu.bitwise_and)
                ts(adb[s][:, 0:g, 2:4, :].bitcast(U16),
                   dxyb[s][:, 0:g, 0:2, :].bitcast(U16),
                   0x7FFF, None, Alu.bitwise_and)
                ts(s35[s][:, 0:g], swh[s][:, 0:g], 3.5, None, Alu.mult)
                ts(mwh7[s][:, 0:g], mwh[s][:, 0:g], 7.0, None, Alu.mult)
                tt(sa[s][:, 0:g], apw[s][:, 0:g],
                   agb[s][:, 0:g].broadcast_to([P, g, 2, K]), Alu.add)
                v.drain()
                # W3  (adb is box-reversed relative to s35: flip its view)
                tt(ta[s][:, 0:g], s35[s][:, 0:g],
                   adb[s][:, 0:g].rearrange("p g (a c) k -> p g a c k", a=2),
                   Alu.subtract)
                ts(sa49[s][:, 0:g], sa[s][:, 0:g], 49.0, None, Alu.mult)
                v.drain()
                tt(ov[s][:, 0:g], mwh7[s][:, 0:g], ta[s][:, 0:g], Alu.min)
                v.drain()
                ts(cl[s][:, 0:g], ov[s][:, 0:g], 0.0, None, Alu.max)
                v.drain()
                tt(ints[s][:, 0:g], cl[s][:, 0:g, :, 0:1, :],
                   cl[s][:, 0:g, :, 1:2, :], Alu.mult)
                v.drain()
                tt(u49[s][:, 0:g], sa49[s][:, 0:g], ints[s][:, 0:g],
                   Alu.subtract)
                v.drain()
                with nc.allow_low_precision(reason="bf16 iou tolerated"):
                    v.reciprocal(rcp[s][:, 0:g], u49[s][:, 0:g])
                v.drain()
                tt(iou[s][:, 0:g], ints[s][:, 0:g], rcp[s][:, 0:g], Alu.mult)
                v.drain()
                # W10: use1 + dconf
                tt(tq[s][:, 0:g, 0:1, :], iou[s][:, 0:g, 0:1, :],
                   iou[s][:, 0:g, 1:2, :], Alu.is_ge)
                tt(dconf[s][:, 0:g], p[:, 0:g, 8:10, :], iou[s][:, 0:g],
                   Alu.subtract)
                v.drain()
                # W10b: tq = linear sqrt-mask factors of u ; dsq
                ts(tq[s][:, 0:g, 1:2, :], tq[s][:, 0:g, 0:1, :],
                   -2.23606798, 2.23606798, Alu.mult, Alu.add)
                ts(tq[s][:, 0:g, 2:3, :], tq[s][:, 0:g, 0:1, :],
                   0.29289322, 0.70710678, Alu.mult, Alu.add)
                ts(tq[s][:, 0:g, 3:4, :], tq[s][:, 0:g, 0:1, :],
                   -0.29289322, 1.0, Alu.mult, Alu.add)
                ts(tq[s][:, 0:g, 0:1, :], tq[s][:, 0:g, 0:1, :],
                   2.23606798, None, Alu.mult)
                v.wait_ge(sqrt_done, i + 1)
                tt(dxyb[s][:, 0:g, 4:6, :], sqp[s][:, 0:g, 0:1, :, :],
                   sql[s][:, 0:g, 0:1, :, :], Alu.subtract)
                tt(dxyb[s][:, 0:g, 8:10, :], sqp[s][:, 0:g, 1:2, :, :],
                   sql[s][:, 0:g, 1:2, :, :],
                   Alu.subtract).then_inc(u_done, 1)
                v.drain().then_inc(tfree_dve, 1)
                if i >= 1 and i != nchunk - 1:
                    lagged_mults(i - 1)
            lagged_mults(nchunk - 1)

    return nc


_NC_CACHE = {}


def _get_nc():
    if "nc" not in _NC_CACHE:
        _NC_CACHE["nc"] = build_nc()
    return _NC_CACHE["nc"]


def _to_bf16_repack(pred, labels):
    import ml_dtypes

    bf = ml_dtypes.bfloat16
    p = np.ascontiguousarray(pred, dtype=np.float32).reshape(B_TOTAL, C, K)
    l = np.ascontiguousarray(labels, dtype=np.float32).reshape(B_TOTAL, C, K)
    pb = p.astype(bf)
    lb = l.astype(bf)
    # obj channel: keep the ==1.0 test exact under rounding
    l4 = l[:, 4, :]
    lb4 = lb[:, 4, :]
    bad = (l4 != np.float32(1.0)) & (lb4.astype(np.float32) == np.float32(1.0))
    if bad.any():
        lb4[bad] = bf(0.99609375)
        lb[:, 4, :] = lb4
    prp = np.ascontiguousarray(pb[:, PP_IDX, :]).reshape(B_TOTAL, ROWP)
    lrp = np.ascontiguousarray(lb[:, LL_IDX, :]).reshape(B_TOTAL, ROWL)
    return prp, lrp


def run_device(pred, labels, trace=False):
    nc = _get_nc()
    prp, lrp = _to_bf16_repack(pred, labels)
    in_maps = []
    for c in range(NCORES):
        rows = slice(c * B_CORE, (c + 1) * B_CORE)
        in_maps.append({"pred": prp[rows], "labels": lrp[rows]})
    res = run_bass_kernel_spmd(nc, in_maps, list(range(NCORES)), trace=trace)
    total = 0.0
    for c in range(NCORES):
        total += float(res.results[c]["out"].astype(np.float64).sum())
    loss = np.float32(total / B_TOTAL)
    return loss, res


def kernel(pred, labels):
    loss, _ = run_device(pred, labels, trace=False)
    return np.array(loss, dtype=np.float32)


if __name__ == "__main__":
    rng = np.random.default_rng(0)
    p = rng.random((B_TOTAL, C, 7, 7), dtype=np.float32)
    l = rng.random((B_TOTAL, C, 7, 7), dtype=np.float32)
    l[:, 4] = (rng.random((B_TOTAL, 7, 7)) < 0.3).astype(np.float32)
    print(kernel(p, l))


# revision 40
# speedup vs baseline: 2.0706x; 1.0475x over previous
"""YOLOv1 loss kernel for Trainium2, 8-core data-parallel, bf16 pipeline.

Strategy: shard batch (8192) across 8 cores (1024 rows each). Host converts
inputs to bf16 (labels obj channel converted equality-preserving so l4 == 1.0
stays exact) and repacks channels per-row so every multi-channel device op is
one contiguous instruction:

  pred row (30 ch):  [0,1,5,6 | 2,3,7,8 | 4,9 | 10..29]
  label row (35 ch): [0,1,0,1 | 2,3,2,3 | 5,6 | 2,3,7,8 | 4 | 10..29]

Each core streams its shard in uneven chunks (g units of 128 rows, layout
[128, g, ch, 49]). Per chunk the whole loss reduces to ONE Scalar-engine
Square+accumulate over a packed masked buffer mball[g, 32, 49]:
  slots 0:8   sqrt(5*om_b) * (coor diffs: dx, dy, sqrt-w, sqrt-h per box)
  slots 8:10  sqrt(mA/mB) * (conf - iou) per box
  slots 10:12 sqrt(0.5*(1-obj)) * (p4, p9)
  slots 12:32 obj * (pred_cls - label_cls)
where om_u = obj*use1, om_nu = obj*(1-use1), mA = om_u + 0.5*om_nu,
mB = om_nu + 0.5*om_u. sum(mball^2) == chunk loss contribution exactly.

IoU runs in 7x-scaled units: ov7 = max(min(7*min(w), 3.5*(wp+wl) - |dc|), 0),
ints49 = ov7w*ov7h, u49 = 49*(area_p + area_g) - ints49, iou = ints49/u49.

All sqrt-mask values are linear in use1 (e.g. sqrt(mA) = 0.7071 + 0.2929*u,
sqrt(5*om_u) = sqrt(5)*obj*u), so the mask vector rm[5] is built with plain
mult/add ops - no activation-engine involvement on the mask path.

Engine split: DVE does diffs/IoU (tensor_tensor 2x bf16, tensor_scalar 4x)
with the masked-multiply wave lagged one chunk behind (software pipelining);
Pool (gpsimd) computes obj, the rm vector, 8 of 20 cls channels and the
masked conf/q multiplies; ACT does the w/h sqrts and one Square+accum_out
per chunk (lagged two chunks so sqrts never queue behind it). Every
chunk's DMA is split into an IoU-channel head and a cls tail so IoU work
starts before the cls channels land; the last chunk computes its own mask
vector on DVE to shorten the tail.
Out: acc[128, NCHUNK] fp32 per core, summed on host in fp64.
"""

import sys

import numpy as np

for _p in ("/opt/trn_rl_repo", "/root/.axon_site/_ro/trn_rl_repo"):
    if _p not in sys.path:
        sys.path.insert(0, _p)

import concourse.bass as bass
import concourse.mybir as mybir
from concourse.bass_utils import run_bass_kernel_spmd

F32 = mybir.dt.float32
BF16 = mybir.dt.bfloat16
U16 = mybir.dt.uint16
Alu = mybir.AluOpType
Act = mybir.ActivationFunctionType

B_TOTAL = 8192
NCORES = 8
B_CORE = B_TOTAL // NCORES  # 1024
P = 128
C = 30
K = 49
CP = 30   # repacked pred channels
CL = 35   # repacked label channels
ROWP = CP * K
ROWL = CL * K

# host channel permutations
PP_IDX = [0, 1, 5, 6, 2, 3, 7, 8, 4, 9] + list(range(10, 30))
LL_IDX = [0, 1, 0, 1, 2, 3, 2, 3, 5, 6, 2, 3, 7, 8, 4] + list(range(10, 30))

CHUNKS = (2, 3, 2, 1)


def build_nc(chunks=CHUNKS):
    assert sum(chunks) * P == B_CORE
    nchunk = len(chunks)
    maxg = max(chunks)
    nc = bass.Bass()
    pred = nc.declare_dram_parameter("pred", [B_CORE, ROWP], BF16, isOutput=False)
    labels = nc.declare_dram_parameter("labels", [B_CORE, ROWL], BF16,
                                       isOutput=False)
    out = nc.declare_dram_parameter("out", [P, nchunk], F32, isOutput=True)

    from contextlib import ExitStack

    _ctr = [0]
    es = ExitStack()

    def sb(shape, dt=BF16):
        _ctr[0] += 1
        return es.enter_context(nc.sbuf_tensor(f"t{_ctr[0]}", shape, dt))

    with es:
        pt = [sb([P, maxg, CP, K]) for _ in range(2)]
        lt = [sb([P, maxg, CL, K]) for _ in range(2)]
        # dxyb slots: 0:2 b2-iou-xy, 2:4 b1-xy, 4:6 b1-sqrt, 6:8 b2-xy, 8:10 b2-sqrt
        dxyb = [sb([P, maxg, 10, K]) for _ in range(2)]
        adb = [sb([P, maxg, 4, K]) for _ in range(2)]     # |b2Ix,b2Iy,b1x,b1y|
        swh = [sb([P, maxg, 2, 2, K]) for _ in range(2)]
        s35 = [sb([P, maxg, 2, 2, K]) for _ in range(2)]
        mwh = [sb([P, maxg, 2, 2, K]) for _ in range(2)]
        mwh7 = [sb([P, maxg, 2, 2, K]) for _ in range(2)]
        ta = [sb([P, maxg, 2, 2, K]) for _ in range(2)]
        ov = [sb([P, maxg, 2, 2, K]) for _ in range(2)]
        cl = [sb([P, maxg, 2, 2, K]) for _ in range(2)]
        ints = [sb([P, maxg, 2, K]) for _ in range(2)]
        apw = [sb([P, maxg, 2, K]) for _ in range(2)]
        agb = [sb([P, maxg, 1, K]) for _ in range(2)]
        sa = [sb([P, maxg, 2, K]) for _ in range(2)]
        sa49 = [sb([P, maxg, 2, K]) for _ in range(2)]
        u49 = [sb([P, maxg, 2, K]) for _ in range(2)]
        rcp = [sb([P, maxg, 2, K]) for _ in range(2)]
        iou = [sb([P, maxg, 2, K]) for _ in range(2)]
        crx = [sb([P, maxg, 2, K]) for _ in range(2)]
        tq = [sb([P, maxg, 5, K]) for _ in range(2)]  # u, then 4 factors
        dconf = [sb([P, maxg, 2, K]) for _ in range(2)]
        dcls = [sb([P, maxg, 12, K]) for _ in range(2)]
        dclsp = [sb([P, maxg, 8, K]) for _ in range(2)]
        sqp = [sb([P, maxg, 2, 2, K]) for _ in range(2)]
        sql = [sb([P, maxg, 2, 2, K]) for _ in range(2)]
        objm = [sb([P, maxg, K]) for _ in range(2)]
        objd = sb([P, maxg, K])
        rm = [sb([P, maxg, 5, K]) for _ in range(2)]
        mball = [sb([P, maxg, 32, K]) for _ in range(2)]
        junk32 = [sb([P, maxg, 32, K]) for _ in range(2)]
        acc = es.enter_context(nc.sbuf_tensor("acc", [P, nchunk], F32))

        dsemA = es.enter_context(nc.semaphore("dsemA"))
        dsemB = es.enter_context(nc.semaphore("dsemB"))
        dsems = [dsemA, dsemB]
        u_done = es.enter_context(nc.semaphore("u_done"))
        sqrt_done = es.enter_context(nc.semaphore("sqrt_done"))
        rm_done = es.enter_context(nc.semaphore("rm_done"))
        mball_dve = es.enter_context(nc.semaphore("mball_dve"))
        mball_pool = es.enter_context(nc.semaphore("mball_pool"))
        acc_done = es.enter_context(nc.semaphore("acc_done"))
        tfree_dve = es.enter_context(nc.semaphore("tfree_dve"))
        area_done = es.enter_context(nc.semaphore("area_done"))
        dsemTA = es.enter_context(nc.semaphore("dsemTA"))
        dsemTB = es.enter_context(nc.semaphore("dsemTB"))
        dsemTs = [dsemTA, dsemTB]
        block = es.enter_context(nc.Block())

        offs = [0]
        for g in chunks:
            offs.append(offs[-1] + g * P)

        # dsem thresholds: chunk 0 is split into head (iou ch) + tail (cls ch)
        head_v = {}
        tail_v = {}
        _dv = [0, 0]
        for i in range(nchunk):
            s = i % 2
            head_v[i] = tail_v[i] = _dv[s] + 32
            _dv[s] += 32

        @block.sync
        def _(sync):
            for i, g in enumerate(chunks):
                s = i % 2
                if i >= 2:
                    sync.wait_ge(sqrt_done, i - 1)
                    sync.wait_ge(mball_pool, i - 1)
                    sync.wait_ge(tfree_dve, i - 1)
                rows = slice(offs[i], offs[i + 1])
                sync.dma_start(
                    out=pt[s][:, 0:g, 0:10, :].rearrange(
                        "p g c k -> p g (c k)"),
                    in_=pred[rows, 0:10 * K].rearrange(
                        "(g p) d -> p g d", p=P),
                ).then_inc(dsems[s], 16)
                sync.dma_start(
                    out=lt[s][:, 0:g, 0:15, :].rearrange(
                        "p g c k -> p g (c k)"),
                    in_=labels[rows, 0:15 * K].rearrange(
                        "(g p) d -> p g d", p=P),
                ).then_inc(dsems[s], 16)
                sync.dma_start(
                    out=pt[s][:, 0:g, 10:30, :].rearrange(
                        "p g c k -> p g (c k)"),
                    in_=pred[rows, 10 * K:].rearrange(
                        "(g p) d -> p g d", p=P),
                ).then_inc(dsemTs[s], 16)
                sync.dma_start(
                    out=lt[s][:, 0:g, 15:35, :].rearrange(
                        "p g c k -> p g (c k)"),
                    in_=labels[rows, 15 * K:].rearrange(
                        "(g p) d -> p g d", p=P),
                ).then_inc(dsemTs[s], 16)
            sync.wait_ge(acc_done, nchunk)
            sync.dma_start(out=out[:], in_=acc[:]).then_inc(dsemA, 16)
            sync.wait_ge(dsemA, _dv[0] + 16)

        @block.gpsimd
        def _(gp):
            for i, g in enumerate(chunks):
                s = i % 2
                gp.wait_ge(dsems[s], head_v[i])
                if i >= 2:
                    gp.wait_ge(mball_dve, i - 1)
                gp.tensor_scalar(objm[s][:, 0:g], lt[s][:, 0:g, 14:15, :],
                                 1.0, None, Alu.is_equal)
                gp.wait_ge(dsemTs[s], tail_v[i])
                gp.tensor_tensor(dclsp[s][:, 0:g], pt[s][:, 0:g, 22:30, :],
                                 lt[s][:, 0:g, 27:35, :], Alu.subtract)
                gp.drain()
                gp.wait_ge(u_done, i + 1)
                if i != nchunk - 1:
                    gp.tensor_scalar(rm[s][:, 0:g, 4:5, :], objm[s][:, 0:g],
                                     -0.70710678, 0.70710678, Alu.mult,
                                     Alu.add)
                    gp.tensor_tensor(
                        rm[s][:, 0:g, 0:4, :],
                        objm[s][:, 0:g].unsqueeze(2).broadcast_to(
                            [P, g, 4, K]),
                        tq[s][:, 0:g, 1:5, :], Alu.mult,
                    ).then_inc(rm_done, 1)
                    gp.drain()
                else:
                    gp.wait_ge(rm_done, nchunk)
                if i >= 2:
                    gp.wait_ge(acc_done, i - 1)
                gp.tensor_tensor(
                    mball[s][:, 0:g, 24:32, :], dclsp[s][:, 0:g],
                    objm[s][:, 0:g].unsqueeze(2).broadcast_to([P, g, 8, K]),
                    Alu.mult,
                )
                gp.tensor_tensor(
                    mball[s][:, 0:g, 10:12, :], pt[s][:, 0:g, 8:10, :],
                    rm[s][:, 0:g, 4:5, :].broadcast_to([P, g, 2, K]),
                    Alu.mult,
                )
                gp.tensor_tensor(
                    mball[s][:, 0:g, 8:10, :], dconf[s][:, 0:g],
                    rm[s][:, 0:g, 2:4, :], Alu.mult,
                ).then_inc(mball_pool, 1)

        @block.scalar
        def _(act):
            def sq_acc(j):
                sj = j % 2
                gj = chunks[j]
                act.wait_ge(mball_dve, j + 1)
                act.wait_ge(mball_pool, j + 1)
                act.activation(
                    junk32[sj][:, 0:gj].rearrange("p g c k -> p (g c k)"),
                    mball[sj][:, 0:gj].rearrange("p g c k -> p (g c k)"),
                    Act.Square,
                    accum_out=acc[:, j:j + 1],
                ).then_inc(acc_done, 1)

            for i, g in enumerate(chunks):
                s = i % 2
                act.wait_ge(dsems[s], head_v[i])
                act.activation(sqp[s][:, 0:g], pt[s][:, 0:g, 4:8, :], Act.Sqrt)
                act.activation(sql[s][:, 0:g], lt[s][:, 0:g, 10:14, :],
                               Act.Sqrt).then_inc(sqrt_done, 1)
                if i >= 2:
                    sq_acc(i - 2)
            sq_acc(nchunk - 2)
            sq_acc(nchunk - 1)

        @block.vector
        def _(v):
            tt = v.tensor_tensor
            ts = v.tensor_scalar

            def lagged_mults(j):
                sj = j % 2
                gj = chunks[j]
                v.wait_ge(rm_done, j + 1)
                if j >= 2:
                    v.wait_ge(acc_done, j - 1)
                tt(mball[sj][:, 0:gj, 0:4, :], dxyb[sj][:, 0:gj, 2:6, :],
                   rm[sj][:, 0:gj, 0:1, :].broadcast_to([P, gj, 4, K]),
                   Alu.mult)
                tt(mball[sj][:, 0:gj, 4:8, :], dxyb[sj][:, 0:gj, 6:10, :],
                   rm[sj][:, 0:gj, 1:2, :].broadcast_to([P, gj, 4, K]),
                   Alu.mult)
                msk = objd if j == nchunk - 1 else objm[sj]
                tt(mball[sj][:, 0:gj, 12:24, :], dcls[sj][:, 0:gj],
                   msk[:, 0:gj].unsqueeze(2).broadcast_to([P, gj, 12, K]),
                   Alu.mult).then_inc(mball_dve, 1)
                v.drain()

            for i, g in enumerate(chunks):
                s = i % 2
                p, l = pt[s], lt[s]
                if i == nchunk - 1 and i >= 1:
                    lagged_mults(i - 1)
                v.wait_ge(dsems[s], head_v[i])
                # W1: reads only tiles
                tt(dxyb[s][:, 0:g, 2:4, :], p[:, 0:g, 0:2, :],
                   l[:, 0:g, 0:2, :], Alu.subtract)
                tt(dxyb[s][:, 0:g, 0:2, :], p[:, 0:g, 2:4, :],
                   l[:, 0:g, 2:4, :], Alu.subtract)
                tt(dxyb[s][:, 0:g, 6:8, :], p[:, 0:g, 2:4, :],
                   l[:, 0:g, 8:10, :], Alu.subtract)
                tt(swh[s][:, 0:g], p[:, 0:g, 4:8, :], l[:, 0:g, 4:8, :],
                   Alu.add)
                tt(mwh[s][:, 0:g], p[:, 0:g, 4:8, :], l[:, 0:g, 4:8, :],
                   Alu.min)
                tt(apw[s][:, 0:g],
                   p[:, 0:g, 4:8, :].rearrange("p g (b w) k -> p g b w k",
                                               b=2)[:, :, :, 0, :],
                   p[:, 0:g, 4:8, :].rearrange("p g (b w) k -> p g b w k",
                                               b=2)[:, :, :, 1, :],
                   Alu.mult)
                tt(agb[s][:, 0:g], l[:, 0:g, 4:5, :], l[:, 0:g, 5:6, :],
                   Alu.mult)
                if i == nchunk - 1:
                    ts(objd[:, 0:g], l[:, 0:g, 14:15, :], 1.0, None,
                       Alu.is_equal)
                v.wait_ge(dsemTs[s], tail_v[i])
                tt(dcls[s][:, 0:g], p[:, 0:g, 10:22, :], l[:, 0:g, 15:27, :],
                   Alu.subtract)
                v.drain()
                # W2
                ts(adb[s][:, 0:g, 0:2, :].bitcast(U16),
                   dxyb[s][:, 0:g, 2:4, :].bitcast(U16),
                   0x7FFF, None, Alu.bitwise_and)
                ts(adb[s][:, 0:g, 2:4, :].bitcast(U16),
                   dxyb[s][:, 0:g, 0:2, :].bitcast(U16),
                   0x7FFF, None, Alu.bitwise_and)
                ts(s35[s][:, 0:g], swh[s][:, 0:g], 3.5, None, Alu.mult)
                ts(mwh7[s][:, 0:g], mwh[s][:, 0:g], 7.0, None, Alu.mult)
                tt(sa[s][:, 0:g], apw[s][:, 0:g],
                   agb[s][:, 0:g].broadcast_to([P, g, 2, K]), Alu.add)
                v.drain()
                # W3  (adb is box-reversed relative to s35: flip its view)
                tt(ta[s][:, 0:g], s35[s][:, 0:g],
                   adb[s][:, 0:g].rearrange("p g (a c) k -> p g a c k", a=2),
                   Alu.subtract)
                ts(sa49[s][:, 0:g], sa[s][:, 0:g], 49.0, None, Alu.mult)
                v.drain()
                tt(ov[s][:, 0:g], mwh7[s][:, 0:g], ta[s][:, 0:g], Alu.min)
                v.drain()
                ts(cl[s][:, 0:g], ov[s][:, 0:g], 0.0, None, Alu.max)
                v.drain()
                tt(ints[s][:, 0:g], cl[s][:, 0:g, :, 0:1, :],
                   cl[s][:, 0:g, :, 1:2, :], Alu.mult)
                v.drain()
                tt(u49[s][:, 0:g], sa49[s][:, 0:g], ints[s][:, 0:g],
                   Alu.subtract)
                v.drain()
                # use1 via cross products (no recip on its path):
                # iou1>=iou2  <=>  n1*d2 >= n2*d1
                tt(crx[s][:, 0:g], ints[s][:, 0:g],
                   u49[s][:, 0:g, 1::-1, :], Alu.mult)
                with nc.allow_low_precision(reason="bf16 iou tolerated"):
                    v.reciprocal(rcp[s][:, 0:g], u49[s][:, 0:g])
                v.drain()
                tt(tq[s][:, 0:g, 0:1, :], crx[s][:, 0:g, 0:1, :],
                   crx[s][:, 0:g, 1:2, :], Alu.is_ge)
                tt(iou[s][:, 0:g], ints[s][:, 0:g], rcp[s][:, 0:g], Alu.mult)
                v.drain()
                # dconf; tq = linear sqrt-mask factors of u ; dsq
                tt(dconf[s][:, 0:g], p[:, 0:g, 8:10, :], iou[s][:, 0:g],
                   Alu.subtract)
                ts(tq[s][:, 0:g, 1:2, :], tq[s][:, 0:g, 0:1, :],
                   2.23606798, None, Alu.mult)
                ts(tq[s][:, 0:g, 2:3, :], tq[s][:, 0:g, 0:1, :],
                   -2.23606798, 2.23606798, Alu.mult, Alu.add)
                ts(tq[s][:, 0:g, 3:4, :], tq[s][:, 0:g, 0:1, :],
                   0.29289322, 0.70710678, Alu.mult, Alu.add)
                ts(tq[s][:, 0:g, 4:5, :], tq[s][:, 0:g, 0:1, :],
                   -0.29289322, 1.0, Alu.mult, Alu.add).then_inc(u_done, 1)
                v.wait_ge(sqrt_done, i + 1)
                tt(dxyb[s][:, 0:g, 4:6, :], sqp[s][:, 0:g, 0:1, :, :],
                   sql[s][:, 0:g, 0:1, :, :], Alu.subtract)
                tt(dxyb[s][:, 0:g, 8:10, :], sqp[s][:, 0:g, 1:2, :, :],
                   sql[s][:, 0:g, 1:2, :, :], Alu.subtract)
                v.drain().then_inc(tfree_dve, 1)
                if i == nchunk - 1:
                    ts(rm[s][:, 0:g, 4:5, :], objd[:, 0:g],
                       -0.70710678, 0.70710678, Alu.mult, Alu.add)
                    tt(rm[s][:, 0:g, 0:4, :],
                       objd[:, 0:g].unsqueeze(2).broadcast_to([P, g, 4, K]),
                       tq[s][:, 0:g, 1:5, :],
                       Alu.mult).then_inc(rm_done, 1)
                    v.drain()
                if i >= 1 and i != nchunk - 1:
                    lagged_mults(i - 1)
            lagged_mults(nchunk - 1)

    return nc


_NC_CACHE = {}


def _get_nc():
    if "nc" not in _NC_CACHE:
        _NC_CACHE["nc"] = build_nc()
    return _NC_CACHE["nc"]


def _to_bf16_repack(pred, labels):
    import ml_dtypes

    bf = ml_dtypes.bfloat16
    p = np.ascontiguousarray(pred, dtype=np.float32).reshape(B_TOTAL, C, K)
    l = np.ascontiguousarray(labels, dtype=np.float32).reshape(B_TOTAL, C, K)
    pb = p.astype(bf)
    lb = l.astype(bf)
    # obj channel: keep the ==1.0 test exact under rounding
    l4 = l[:, 4, :]
    lb4 = lb[:, 4, :]
    bad = (l4 != np.float32(1.0)) & (lb4.astype(np.float32) == np.float32(1.0))
    if bad.any():
        lb4[bad] = bf(0.99609375)
        lb[:, 4, :] = lb4
    prp = np.ascontiguousarray(pb[:, PP_IDX, :]).reshape(B_TOTAL, ROWP)
    lrp = np.ascontiguousarray(lb[:, LL_IDX, :]).reshape(B_TOTAL, ROWL)
    return prp, lrp


def run_device(pred, labels, trace=False):
    nc = _get_nc()
    prp, lrp = _to_bf16_repack(pred, labels)
    in_maps = []
    for c in range(NCORES):
        rows = slice(c * B_CORE, (c + 1) * B_CORE)
        in_maps.append({"pred": prp[rows], "labels": lrp[rows]})
    res = run_bass_kernel_spmd(nc, in_maps, list(range(NCORES)), trace=trace)
    total = 0.0
    for c in range(NCORES):
        total += float(res.results[c]["out"].astype(np.float64).sum())
    loss = np.float32(total / B_TOTAL)
    return loss, res


def kernel(pred, labels):
    loss, _ = run_device(pred, labels, trace=False)
    return np.array(loss, dtype=np.float32)


if __name__ == "__main__":
    rng = np.random.default_rng(0)
    p = rng.random((B_TOTAL, C, 7, 7), dtype=np.float32)
    l = rng.random((B_TOTAL, C, 7, 7), dtype=np.float32)
    l[:, 4] = (rng.random((B_TOTAL, 7, 7)) < 0.3).astype(np.float32)
    print(kernel(p, l))
